# revision 14
# baseline (speedup 1.0000x reference)
"""Trainium2 Bass kernel for nn_DiffuRNNLayer (B=8, N=2048, D=1024).

Sharding: data-parallel over batch - one batch element per NeuronCore (8 cores).
Per-core phases:
  A: Q/K/V projections in fp8 DoubleRow (+elu+1); Qp (bf16, scaled by WS),
     Kp (fp8, x16) / V (fp8) SBUF-resident; K_sum accumulated inline.
  B: KV = Kp^T V via fp8 DoubleRow from SBUF-resident Kp/V.
  C: acc = dwconv''(x) + MLP(x) + tokenmixer(LN(x)) in fp16; spill acc.
  D: attn numerator with 1/norm folded into Qp, acc += attn; LN1; FFN
     residual; LN2; write y^T (fp16).
"""

import numpy as np
import ml_dtypes
from contextlib import ExitStack

import concourse.bass as bass
import concourse.bacc as bacc
import concourse.tile as tile
import concourse.mybir as mybir
from concourse.bass_utils import run_bass_kernel_spmd

F32 = mybir.dt.float32
BF16 = mybir.dt.bfloat16
F16 = mybir.dt.float16
F8 = mybir.dt.float8e4
AF = mybir.ActivationFunctionType
OP = mybir.AluOpType
DR = mybir.MatmulPerfMode.DoubleRow
BF16_NP = ml_dtypes.bfloat16
F16_NP = np.float16
F8_NP = ml_dtypes.float8_e4m3

P = 128
D = 1024
DO = D // P  # 8 chunks of the channel dim
WS = 256.0   # fp8 weight scale for wq/wv
KS = 16.0    # fp8 weight scale for wk (kp8 = KS*Kp must stay under 240)
LNWS = float(np.log(WS))
LNKS = float(np.log(KS))

# pp param-plane indices (per-partition params, laid out [128, DO, NP])
(C0, C1, C2, CB, T0, T1, T2, TCB1, U0, U1, U2,
 TMG, TMB, N1G, N1B, N2G, N2B, LUB1, FFB1, FFB2) = range(20)
NPARAM = 20


def build_nc(N=2048, NT=512, use_bq=False, use_bk=False, use_bv=False,
             use_tmb=False, use_n1b=False, use_n2b=False, debug=False):
    NTILES = N // NT
    NTA = 1024             # phase-A supertile width
    NST = N // NTA
    NCH_A = NTA // P       # 128-token chunks per supertile
    TOTCH = N // P
    W = NT + 4             # phase-C tile width with +-2 halo
    assert N % NT == 0 and NT % P == 0 and N % NTA == 0

    nc = bacc.Bacc(None, target_bir_lowering=False, debug=debug)

    xT_d = nc.dram_tensor("x_T", [D, N], F16, kind="ExternalInput")
    x8_d = nc.dram_tensor("x_8", [D, N], F8, kind="ExternalInput")
    w8_d = {}
    for name in ("wq8", "wk8", "wv8"):
        w8_d[name] = nc.dram_tensor(name, [D, D], F8, kind="ExternalInput")
    w_d = {}
    for name in ("w1T", "w2T", "f1T", "f2T"):
        w_d[name] = nc.dram_tensor(name, [D, D], F16, kind="ExternalInput")
    pp_d = nc.dram_tensor("pp", [P, DO, NPARAM], F32, kind="ExternalInput")
    diags_d = nc.dram_tensor("diags", [P, 3, DO, P], F16, kind="ExternalInput")
    rows_d = nc.dram_tensor("rows", [1, 3 * D], BF16, kind="ExternalInput")
    yT_d = nc.dram_tensor("y_T", [D, N], F16, kind="ExternalOutput")

    acc_sp = nc.dram_tensor("acc_sp", [D, N], F16)

    xT = xT_d.rearrange("(o p) n -> p o n", p=P)
    x8r = x8_d.rearrange("(o p) n -> p o n", p=P)
    w8r = {k: v.rearrange("(o p) n -> p o n", p=P) for k, v in w8_d.items()}
    wr = {k: v.rearrange("(o p) n -> p o n", p=P) for k, v in w_d.items()}
    acc_r = acc_sp.rearrange("(o p) n -> p o n", p=P)
    yT = yT_d.rearrange("(o p) n -> p o n", p=P)

    with tile.TileContext(nc) as tc, ExitStack() as top:
        persist = top.enter_context(tc.tile_pool(name="persist", bufs=1))
        ones_1p_f32 = persist.tile([1, P], F32)
        nc.vector.memset(ones_1p_f32, 1.0)
        ones_one = persist.tile([1, 1], BF16)
        nc.vector.memset(ones_one, 1.0)
        ones8 = persist.tile([P, 2, 16], F8)
        nc.vector.memset(ones8, 1.0)
        ksrow_sb = persist.tile([1, D], BF16)
        onesD = persist.tile([P, P], F16)
        nc.vector.memset(onesD, 1.0 / D)
        eps_ln = persist.tile([P, 1], F32)
        nc.vector.memset(eps_ln, 1e-5)
        lnws = persist.tile([P, 1], F32)
        nc.vector.memset(lnws, LNWS)
        lnks = persist.tile([P, 1], F32)
        nc.vector.memset(lnks, LNKS)
        kv_sb = persist.tile([P, DO, D], BF16)
        ksum_sb = persist.tile([P, DO, 1], BF16)
        qp = persist.tile([P, DO, N], BF16)  # WS-scaled Qp, resident
        pp = persist.tile([P, DO, NPARAM], F32)
        diags = persist.tile([P, 3, DO, P], F16)
        rows = ones_row = ones_1p_bf = None
        if use_bq or use_bk or use_bv:
            rows = persist.tile([1, 3 * D], BF16)
            ones_row = persist.tile([1, NTA], BF16)
            nc.vector.memset(ones_row, 1.0)
            ones_1p_bf = persist.tile([1, P], BF16)
            nc.vector.memset(ones_1p_bf, 1.0)

        def stats_mm(psum, lhs_ones, rhs3, width):
            """Accumulate over DO k-chunks: psum[:, j] = mean over channel dim,
            replicated across partitions.  rhs3: [P, DO, width]."""
            for c0 in range(0, width, 512):
                cw = min(512, width - c0)
                for kc in range(DO):
                    nc.tensor.matmul(psum[:, c0:c0 + cw], lhs_ones,
                                     rhs3[:, kc, c0:c0 + cw],
                                     start=(kc == 0), stop=(kc == DO - 1))

        # ---------------- Phases A+B: QKV + KV (fp8 DoubleRow) ----------------
        with ExitStack() as ph:
            wpool = ph.enter_context(tc.tile_pool(name="wA", bufs=1))
            wq_sb = wpool.tile([P, DO, D], F8, tag="wq")
            nc.sync.dma_start(wq_sb, w8r["wq8"])
            kvres = ph.enter_context(tc.tile_pool(name="kvres", bufs=1))
            kp8 = kvres.tile([P, TOTCH, D], F8, tag="kp8")
            v8 = kvres.tile([P, TOTCH, D], F8, tag="v8")
            io = ph.enter_context(tc.tile_pool(name="ioA", bufs=2))
            x8_0 = io.tile([P, DO, NTA], F8, tag="x8A", name="x8_0")
            nc.sync.dma_start(x8_0, x8r[:, :, 0:NTA])
            wk_sb = wpool.tile([P, DO, D], F8, tag="wk")
            nc.sync.dma_start(wk_sb, w8r["wk8"])
            wv_sb = wpool.tile([P, DO, D], F8, tag="wv")
            nc.sync.dma_start(wv_sb, w8r["wv8"])
            if use_bq or use_bk or use_bv:
                nc.sync.dma_start(rows, rows_d[:])
            nc.sync.dma_start(pp, pp_d[:])
            nc.sync.dma_start(diags, diags_d[:])

            ev = ph.enter_context(tc.tile_pool(name="evA", bufs=4))
            pa = ph.enter_context(ExitStack())
            psQ = pa.enter_context(tc.tile_pool(name="psQA", bufs=1, space="PSUM"))
            psK = pa.enter_context(tc.tile_pool(name="psKA", bufs=2, space="PSUM"))
            psV = pa.enter_context(tc.tile_pool(name="psVA", bufs=2, space="PSUM"))
            psKS = pa.enter_context(tc.tile_pool(name="psKSA", bufs=1, space="PSUM"))
            ps_ks = [psKS.tile([1, 512], F32, tag=f"ksr{h}", name=f"ksr{h}")
                     for h in range(2)]

            def ksum_pair(chp):
                c2 = slice(2 * chp, 2 * chp + 2)
                for h in range(2):
                    nc.tensor.matmul(ps_ks[h], ones8[:, :, 0:1],
                                     kp8[:, c2, h * 512:(h + 1) * 512],
                                     start=(chp == 0), stop=(chp == TOTCH // 2 - 1),
                                     perf_mode=DR)

            for st in range(NST):
                n0 = st * NTA
                if st == 0:
                    x8_t = x8_0
                else:
                    x8_t = io.tile([P, DO, NTA], F8, tag="x8A", name=f"x8_{st}")
                    nc.sync.dma_start(x8_t, x8r[:, :, n0:n0 + NTA])

                # ---- Q: layout B, out [dout-chunk, n], WS-scaled bf16 ----
                for dc in range(DO):
                    ps_q = psQ.tile([P, NTA], F32, tag="psq", name=f"psq{st}_{dc}")
                    for kcp in range(DO // 2):
                        ks2 = slice(2 * kcp, 2 * kcp + 2)
                        for h in range(2):
                            hs = slice(h * 512, (h + 1) * 512)
                            nc.tensor.matmul(ps_q[:, hs],
                                             wq_sb[:, ks2, dc * P:(dc + 1) * P],
                                             x8_t[:, ks2, hs],
                                             start=(kcp == 0),
                                             stop=(kcp == DO // 2 - 1 and not use_bq),
                                             perf_mode=DR)
                    if use_bq:
                        for h in range(2):
                            hs = slice(h * 512, (h + 1) * 512)
                            nc.tensor.matmul(ps_q[:, hs],
                                             rows[0:1, dc * P:(dc + 1) * P],
                                             ones_row[0:1, hs], start=False,
                                             stop=True)
                    for h in range(2):
                        hs = slice(h * 512, (h + 1) * 512)
                        # e_all = WS*exp(q); e1 = min(e_all, WS) = WS*exp(min(q,0))
                        e_all = ev.tile([P, 512], BF16, tag="eQ")
                        nc.scalar.activation(e_all, ps_q[:, hs], AF.Exp,
                                             scale=1.0 / WS, bias=lnws[:, 0:1])
                        e1 = ev.tile([P, 512], BF16, tag="e1Q")
                        nc.vector.tensor_scalar_min(e1, e_all, float(WS))
                        # qp' = max(WS*q, 0) + WS*exp(min(q,0)) = WS*Qp
                        nc.vector.scalar_tensor_tensor(
                            qp[:, dc, n0 + h * 512:n0 + (h + 1) * 512],
                            ps_q[:, hs], 0.0, e1, OP.max, OP.add)

                # ---- K, V: layout A, out [token-chunk, dout], fp8 ----
                for ch in range(NCH_A):
                    cs = slice(ch * P, (ch + 1) * P)
                    chg = st * NCH_A + ch
                    # interleave K_sum for the pair finished two chunks ago
                    if chg >= 2 and chg % 2 == 0:
                        ksum_pair(chg // 2 - 1)
                    for h in range(2):
                        hs = slice(h * 512, (h + 1) * 512)
                        ps_k = psK.tile([P, 512], F32, tag="psk")
                        ps_v = psV.tile([P, 512], F32, tag="psv")
                        for kcp in range(DO // 2):
                            ks2 = slice(2 * kcp, 2 * kcp + 2)
                            nc.tensor.matmul(ps_k, x8_t[:, ks2, cs],
                                             wk_sb[:, ks2, hs],
                                             start=(kcp == 0),
                                             stop=(kcp == DO // 2 - 1 and not use_bk),
                                             perf_mode=DR)
                            nc.tensor.matmul(ps_v, x8_t[:, ks2, cs],
                                             wv_sb[:, ks2, hs],
                                             start=(kcp == 0),
                                             stop=(kcp == DO // 2 - 1 and not use_bv),
                                             perf_mode=DR)
                        if use_bk:
                            nc.tensor.matmul(ps_k, ones_1p_bf[0:1, :],
                                             rows[0:1, D + h * 512:D + (h + 1) * 512],
                                             start=False, stop=True)
                        if use_bv:
                            nc.tensor.matmul(ps_v, ones_1p_bf[0:1, :],
                                             rows[0:1, 2 * D + h * 512:2 * D + (h + 1) * 512],
                                             start=False, stop=True)
                        # kp8 = KS*Kp = max(KS*k, 0) + min(KS*e^k, KS)
                        ek = ev.tile([P, 512], BF16, tag="eK")
                        nc.scalar.activation(ek, ps_k, AF.Exp,
                                             scale=1.0 / KS, bias=lnks[:, 0:1])
                        e1k = ev.tile([P, 512], BF16, tag="e1K")
                        nc.vector.tensor_scalar_min(e1k, ek, float(KS))
                        nc.vector.scalar_tensor_tensor(kp8[:, chg, hs], ps_k, 0.0,
                                                       e1k, OP.max, OP.add)
                        nc.scalar.activation(v8[:, chg, hs], ps_v, AF.Copy,
                                             scale=1.0 / WS)

            # drain the last K_sum pair (pairs 0..TOTCH//2-2 issued inline)
            ksum_pair(TOTCH // 2 - 1)
            for h in range(2):
                hs = slice(h * 512, (h + 1) * 512)
                nc.scalar.activation(ksrow_sb[0:1, hs], ps_ks[h], AF.Copy,
                                     scale=1.0 / KS)
            pa.close()
            # transpose K_sum row -> per-partition column layout [P, DO]
            pk = ph.enter_context(ExitStack())
            psks = pk.enter_context(tc.tile_pool(name="psks", bufs=1, space="PSUM"))
            ps_ksc = psks.tile([P, DO], F32, tag="kscol")
            for dc in range(DO):
                nc.tensor.matmul(ps_ksc[:, dc:dc + 1],
                                 ksrow_sb[0:1, dc * P:(dc + 1) * P],
                                 ones_one[0:1, 0:1], start=True, stop=True)
            nc.scalar.activation(ksum_sb[:, :, 0], ps_ksc, AF.Copy)

            # ---------------- Phase B: KV accumulation (fp8 DR) ----------------
            pk.close()
            psB = ph.enter_context(tc.tile_pool(name="psB", bufs=1, space="PSUM"))
            for pass_ in range(2):
                kv_ps = [psB.tile([P, NTA], F32, tag=f"kvps{i}", name=f"kvps{pass_}_{i}")
                         for i in range(4)]
                for chp in range(TOTCH // 2):
                    c2 = slice(2 * chp, 2 * chp + 2)
                    for i in range(4):
                        dc = pass_ * 4 + i
                        for h in range(2):
                            hs = slice(h * 512, (h + 1) * 512)
                            nc.tensor.matmul(kv_ps[i][:, hs],
                                             kp8[:, c2, dc * P:(dc + 1) * P],
                                             v8[:, c2, hs],
                                             start=(chp == 0),
                                             stop=(chp == TOTCH // 2 - 1),
                                             perf_mode=DR)
                for i in range(4):
                    nc.scalar.activation(kv_sb[:, pass_ * 4 + i, :], kv_ps[i],
                                         AF.Copy, scale=1.0 / KS)

        # ---------------- Phase C: conv'' + local MLP + token mixer ----------------
        with ExitStack() as ph:
            wpool = ph.enter_context(tc.tile_pool(name="wC", bufs=1))
            w1_sb = wpool.tile([P, DO, D], F16, tag="w1")
            nc.sync.dma_start(w1_sb, wr["w1T"])
            w2_sb = wpool.tile([P, DO, D], F16, tag="w2")
            nc.sync.dma_start(w2_sb, wr["w2T"])
            io = ph.enter_context(tc.tile_pool(name="ioC", bufs=2))
            pipe = ph.enter_context(tc.tile_pool(name="pipeC", bufs=2))
            mid = ph.enter_context(tc.tile_pool(name="midC", bufs=1))
            sm = ph.enter_context(tc.tile_pool(name="smC", bufs=1))
            ps = ph.enter_context(tc.tile_pool(name="psC", bufs=2, space="PSUM"))
            pst = ph.enter_context(tc.tile_pool(name="pstC", bufs=1, space="PSUM"))

            def c_front(it):
                n0 = it * NT
                x_t = io.tile([P, DO, W], F16, tag="xC", name=f"x_{it}")
                lo, hi = n0 - 2, n0 + NT + 2
                if lo < 0:
                    nc.vector.memset(x_t[:, :, 0:2], 0.0)
                    nc.sync.dma_start(x_t[:, :, 2:W], xT[:, :, 0:hi])
                elif hi > N:
                    nc.vector.memset(x_t[:, :, W - 2:W], 0.0)
                    nc.sync.dma_start(x_t[:, :, 0:W - 2], xT[:, :, lo:N])
                else:
                    nc.sync.dma_start(x_t, xT[:, :, lo:hi])

                dcv = io.tile([P, DO, NT], F16, tag="dcvC", name=f"dcv_{it}")
                # diffusion dwconv'': center tap on ACT, side taps on DVE
                for o in range(DO):
                    nc.scalar.activation(dcv[:, o, :], x_t[:, o, 2:NT + 2],
                                         AF.Identity, bias=pp[:, o, CB:CB + 1],
                                         scale=pp[:, o, C1:C1 + 1])
                for o in range(DO):
                    nc.vector.scalar_tensor_tensor(dcv[:, o, :], x_t[:, o, 1:NT + 1],
                                                   pp[:, o, C0:C0 + 1], dcv[:, o, :],
                                                   OP.mult, OP.add)
                for o in range(DO):
                    nc.vector.scalar_tensor_tensor(dcv[:, o, :], x_t[:, o, 3:NT + 3],
                                                   pp[:, o, C2:C2 + 1], dcv[:, o, :],
                                                   OP.mult, OP.add)

                # local MLP first half
                h1_t = pipe.tile([P, DO, NT], F16, tag="h1", name=f"h1_{it}")
                for dc in range(DO):
                    ps_h = ps.tile([P, NT], F32, tag="psh1", name=f"psh1_{it}_{dc}")
                    for kc in range(DO):
                        nc.tensor.matmul(ps_h, w1_sb[:, kc, dc * P:(dc + 1) * P],
                                         x_t[:, kc, 2:NT + 2],
                                         start=(kc == 0), stop=(kc == DO - 1))
                    nc.scalar.activation(h1_t[:, dc, :], ps_h, AF.Gelu,
                                         bias=pp[:, dc, LUB1:LUB1 + 1])

                # token mixer LN stats
                sq_t = mid.tile([P, DO, W], F16, tag="tokA", name=f"sq_{it}")
                nc.scalar.activation(sq_t, x_t, AF.Square)
                ps_m = pst.tile([P, W], F32, tag="psm", name=f"psm_{it}")
                stats_mm(ps_m, onesD, x_t, W)
                ps_s = pst.tile([P, W], F32, tag="pss", name=f"pss_{it}")
                stats_mm(ps_s, onesD, sq_t, W)
                m_sb = sm.tile([P, W], F16, tag="msb", name=f"msb_{it}")
                nc.scalar.activation(m_sb, ps_m, AF.Copy)
                var = sm.tile([P, W], F32, tag="var", name=f"var_{it}")
                nc.scalar.activation(var, ps_m, AF.Square)
                nc.vector.tensor_sub(var, ps_s, var)
                nc.scalar.activation(var, var, AF.Sqrt, bias=eps_ln[:, 0:1])
                nc.vector.reciprocal_approx_fast(out=var, in_=var)
                rstd = sm.tile([P, W], F16, tag="rstd", name=f"rstd_{it}")
                nc.vector.tensor_copy(rstd, var)
                u_t = mid.tile([P, DO, W], F16, tag="tokA", name=f"u_{it}")
                for o in range(DO):
                    nc.vector.tensor_sub(u_t[:, o, :], x_t[:, o, :], m_sb)
                xm_t = mid.tile([P, DO, W], F16, tag="tokC", name=f"xm_{it}")
                for o in range(DO):
                    nc.vector.scalar_tensor_tensor(xm_t[:, o, :], u_t[:, o, :],
                                                   pp[:, o, TMG:TMG + 1], rstd,
                                                   OP.mult, OP.mult)
                if use_tmb:
                    for o in range(DO):
                        nc.vector.tensor_scalar_add(xm_t[:, o, :], xm_t[:, o, :],
                                                    pp[:, o, TMB:TMB + 1])
                # conv1: t_s[k] = conv1(xm)[k+1], k in [0, W-2)
                t_t = mid.tile([P, DO, W - 2], F16, tag="tokD", name=f"t_{it}")
                for o in range(DO):
                    nc.scalar.activation(t_t[:, o, :], xm_t[:, o, 1:W - 1],
                                         AF.Identity, bias=pp[:, o, TCB1:TCB1 + 1],
                                         scale=pp[:, o, T1:T1 + 1])
                for o in range(DO):
                    nc.vector.scalar_tensor_tensor(t_t[:, o, :], xm_t[:, o, 0:W - 2],
                                                   pp[:, o, T0:T0 + 1],
                                                   t_t[:, o, :], OP.mult, OP.add)
                for o in range(DO):
                    nc.vector.scalar_tensor_tensor(t_t[:, o, :], xm_t[:, o, 2:W],
                                                   pp[:, o, T2:T2 + 1],
                                                   t_t[:, o, :], OP.mult, OP.add)
                t2_t = pipe.tile([P, DO, W - 2], F16, tag="t2", name=f"t2_{it}")
                nc.scalar.activation(t2_t, t_t, AF.Gelu)
                if it == 0:
                    nc.vector.memset(t2_t[:, :, 0:1], 0.0)
                if it == NTILES - 1:
                    nc.vector.memset(t2_t[:, :, W - 3:W - 2], 0.0)
                return x_t, dcv, h1_t, t2_t

            def c_back(it, tiles):
                n0 = it * NT
                x_t, dcv, h1_t, t2_t = tiles
                acc = io.tile([P, DO, NT], F16, tag="accC", name=f"acc_{it}")
                for dc in range(DO):
                    ps_h = ps.tile([P, NT], F32, tag="psh2", name=f"psh2_{it}_{dc}")
                    for kc in range(DO):
                        nc.tensor.matmul(ps_h, w2_sb[:, kc, dc * P:(dc + 1) * P],
                                         h1_t[:, kc, :],
                                         start=(kc == 0), stop=False)
                    for tap in range(3):
                        nc.tensor.matmul(ps_h, diags[:, tap, dc, :],
                                         t2_t[:, dc, tap:NT + tap],
                                         start=False, stop=(tap == 2))
                    nc.vector.scalar_tensor_tensor(acc[:, dc, :], ps_h, 1.0,
                                                   dcv[:, dc, :], OP.mult, OP.add)
                nc.sync.dma_start(acc_r[:, :, n0:n0 + NT], acc)

            pend = {0: c_front(0)}
            for it in range(NTILES):
                if it + 1 < NTILES:
                    pend[it + 1] = c_front(it + 1)
                c_back(it, pend.pop(it))

        # ---------------- Phase D: attention + LN1 + FFN + LN2 ----------------
        # Pipelined; numerator of tile t+1 is split into two half-groups that
        # are issued under tile t's two DVE-bound LN chains.
        with ExitStack() as ph:
            wpoolD = ph.enter_context(tc.tile_pool(name="wD", bufs=1))
            f1_sb = wpoolD.tile([P, DO, D], F16, tag="f1")
            nc.sync.dma_start(f1_sb, wr["f1T"])
            f2_sb = wpoolD.tile([P, DO, D], F16, tag="f2")
            nc.sync.dma_start(f2_sb, wr["f2T"])
            io = ph.enter_context(tc.tile_pool(name="ioD", bufs=3))
            mid = ph.enter_context(tc.tile_pool(name="midD", bufs=1))
            sm = ph.enter_context(tc.tile_pool(name="smD", bufs=2))
            ps = ph.enter_context(tc.tile_pool(name="psD", bufs=2, space="PSUM"))
            psf_pool = ph.enter_context(tc.tile_pool(name="psfD", bufs=3, space="PSUM"))
            pst = ph.enter_context(tc.tile_pool(name="pstD", bufs=1, space="PSUM"))

            def d_load(it):
                n0 = it * NT
                acc_t = io.tile([P, DO, NT], F16, tag="accD", name=f"accD_{it}")
                nc.sync.dma_start(acc_t, acc_r[:, :, n0:n0 + NT])
                return acc_t

            def d_front_a(it, acc_t):
                """norm row, 1/norm fold, numerator halves 0-3."""
                n0 = it * NT
                ps_n = pst.tile([P, NT], F32, tag="psrep", name=f"psn_{it}")
                for kc in range(DO):
                    nc.tensor.matmul(ps_n[0:1, :], ksum_sb[:, kc, :],
                                     qp[:, kc, n0:n0 + NT],
                                     start=(kc == 0), stop=(kc == DO - 1))
                nr = sm.tile([1, NT], F32, tag="nrD", name=f"nr_{it}")
                nc.vector.tensor_scalar_add(nr, ps_n[0:1, :], 1e-6)
                rr = sm.tile([1, NT], F32, tag="rrD", name=f"rr_{it}")
                nc.vector.reciprocal_approx_fast(out=rr, in_=nr)
                ps_rep = pst.tile([P, NT], F32, tag="psrep", name=f"psrep_{it}")
                nc.tensor.matmul(ps_rep, ones_1p_f32[0:1, :], rr, start=True,
                                 stop=True)
                rep_sb = mid.tile([P, NT], BF16, tag="repsb", name=f"rep_{it}")
                nc.scalar.activation(rep_sb, ps_rep, AF.Copy)
                for kc in range(DO):
                    nc.vector.tensor_mul(qp[:, kc, n0:n0 + NT],
                                         qp[:, kc, n0:n0 + NT], rep_sb)
                for ec in range(DO // 2):
                    ps_u = ps.tile([P, NT], F32, tag="psnum", name=f"psnum_{it}_{ec}")
                    for kc in range(DO):
                        nc.tensor.matmul(ps_u, kv_sb[:, kc, ec * P:(ec + 1) * P],
                                         qp[:, kc, n0:n0 + NT],
                                         start=(kc == 0), stop=(kc == DO - 1))
                    nc.vector.tensor_add(acc_t[:, ec, :], acc_t[:, ec, :], ps_u)
                return acc_t

            def d_front_b(it, acc_t):
                n0 = it * NT
                for ec in range(DO // 2, DO):
                    ps_u = ps.tile([P, NT], F32, tag="psnum", name=f"psnum_{it}_{ec}")
                    for kc in range(DO):
                        nc.tensor.matmul(ps_u, kv_sb[:, kc, ec * P:(ec + 1) * P],
                                         qp[:, kc, n0:n0 + NT],
                                         start=(kc == 0), stop=(kc == DO - 1))
                    nc.vector.tensor_add(acc_t[:, ec, :], acc_t[:, ec, :], ps_u)
                return acc_t

            def d_mid(it, acc_t):
                """LN1 stats + apply -> y1 (fp16)."""
                sq_t = mid.tile([P, DO, NT], F16, tag="sqD", name=f"sqD_{it}")
                nc.scalar.activation(sq_t, acc_t, AF.Square)
                ps_m1 = pst.tile([P, NT], F32, tag="psm1", name=f"psm1_{it}")
                stats_mm(ps_m1, onesD, acc_t, NT)
                ps_s1 = pst.tile([P, NT], F32, tag="pss1", name=f"pss1_{it}")
                stats_mm(ps_s1, onesD, sq_t, NT)
                m1_sb = sm.tile([P, NT], F16, tag="m1sb", name=f"m1_{it}")
                nc.scalar.activation(m1_sb, ps_m1, AF.Copy)
                var1 = sm.tile([P, NT], F32, tag="varD", name=f"var1_{it}")
                nc.scalar.activation(var1, ps_m1, AF.Square)
                nc.vector.tensor_sub(var1, ps_s1, var1)
                nc.scalar.activation(var1, var1, AF.Sqrt, bias=eps_ln[:, 0:1])
                nc.vector.reciprocal_approx_fast(out=var1, in_=var1)
                rstd1 = sm.tile([P, NT], F16, tag="rstdb", name=f"rstdb_{it}")
                nc.vector.tensor_copy(rstd1, var1)
                u1_t = mid.tile([P, DO, NT], F16, tag="u1", name=f"u1_{it}")
                for o in range(DO):
                    nc.vector.tensor_sub(u1_t[:, o, :], acc_t[:, o, :], m1_sb)
                y1_t = mid.tile([P, DO, NT], F16, tag="y1", name=f"y1_{it}")
                for o in range(DO):
                    nc.vector.scalar_tensor_tensor(y1_t[:, o, :], u1_t[:, o, :],
                                                   pp[:, o, N1G:N1G + 1], rstd1,
                                                   OP.mult, OP.mult)
                if use_n1b:
                    for o in range(DO):
                        nc.vector.tensor_scalar_add(y1_t[:, o, :], y1_t[:, o, :],
                                                    pp[:, o, N1B:N1B + 1])
                return y1_t

            def d_ffn(it, y1_t):
                f1h_t = mid.tile([P, DO, NT], F16, tag="f1h", name=f"f1h_{it}")
                for dc in range(DO):
                    ps_f = psf_pool.tile([P, NT], F32, tag="psf",
                                         name=f"psf1_{it}_{dc}")
                    for kc in range(DO):
                        nc.tensor.matmul(ps_f, f1_sb[:, kc, dc * P:(dc + 1) * P],
                                         y1_t[:, kc, :],
                                         start=(kc == 0), stop=(kc == DO - 1))
                    nc.scalar.activation(f1h_t[:, dc, :], ps_f, AF.Gelu,
                                         bias=pp[:, dc, FFB1:FFB1 + 1])
                y2_t = mid.tile([P, DO, NT], F16, tag="y2", name=f"y2_{it}")
                for dc in range(DO):
                    ps_f = psf_pool.tile([P, NT], F32, tag="psf",
                                         name=f"psf2_{it}_{dc}")
                    for kc in range(DO):
                        nc.tensor.matmul(ps_f, f2_sb[:, kc, dc * P:(dc + 1) * P],
                                         f1h_t[:, kc, :],
                                         start=(kc == 0), stop=(kc == DO - 1))
                    nc.vector.scalar_tensor_tensor(y2_t[:, dc, :], ps_f,
                                                   pp[:, dc, FFB2:FFB2 + 1],
                                                   y1_t[:, dc, :], OP.add, OP.add)
                return y2_t

            def d_back(it, y2_t):
                n0 = it * NT
                sq2_t = mid.tile([P, DO, NT], F16, tag="sqD", name=f"sq2_{it}")
                nc.scalar.activation(sq2_t, y2_t, AF.Square)
                ps_m2 = pst.tile([P, NT], F32, tag="psm1", name=f"psm2_{it}")
                stats_mm(ps_m2, onesD, y2_t, NT)
                ps_s2 = pst.tile([P, NT], F32, tag="pss1", name=f"pss2_{it}")
                stats_mm(ps_s2, onesD, sq2_t, NT)
                m2_sb = sm.tile([P, NT], F32, tag="m2sb", name=f"m2_{it}")
                nc.scalar.activation(m2_sb, ps_m2, AF.Copy)
                var2 = sm.tile([P, NT], F32, tag="varD", name=f"var2_{it}")
                nc.scalar.activation(var2, ps_m2, AF.Square)
                nc.vector.tensor_sub(var2, ps_s2, var2)
                nc.scalar.activation(var2, var2, AF.Sqrt, bias=eps_ln[:, 0:1])
                nc.vector.reciprocal_approx_fast(out=var2, in_=var2)
                yo_t = mid.tile([P, DO, NT], F16, tag="yo", name=f"yo_{it}")
                for o in range(DO):
                    nc.vector.tensor_sub(yo_t[:, o, :], y2_t[:, o, :], m2_sb)
                for o in range(DO):
                    nc.vector.scalar_tensor_tensor(yo_t[:, o, :], yo_t[:, o, :],
                                                   pp[:, o, N2G:N2G + 1], var2,
                                                   OP.mult, OP.mult)
                if use_n2b:
                    for o in range(DO):
                        nc.vector.tensor_scalar_add(yo_t[:, o, :], yo_t[:, o, :],
                                                    pp[:, o, N2B:N2B + 1])
                nc.sync.dma_start(yT[:, :, n0:n0 + NT], yo_t)

            acc_tiles = {0: d_load(0)}
            if NTILES > 1:
                acc_tiles[1] = d_load(1)
            acc_cur = d_front_b(0, d_front_a(0, acc_tiles.pop(0)))
            acc_nxt = None
            for it in range(NTILES):
                y1_cur = d_mid(it, acc_cur)
                if it + 2 < NTILES:
                    acc_tiles[it + 2] = d_load(it + 2)
                if it + 1 < NTILES:
                    acc_nxt = d_front_a(it + 1, acc_tiles.pop(it + 1))
                y2_cur = d_ffn(it, y1_cur)
                if it + 1 < NTILES:
                    acc_cur = d_front_b(it + 1, acc_nxt)
                d_back(it, y2_cur)

    nc.compile()
    return nc


def make_in_maps(inputs, n_cores=8):
    """Host-side preprocessing: fold constants, transpose, cast, shard."""
    x = np.asarray(inputs["x"], np.float32)
    B, N, D_ = x.shape
    dt = float(np.asarray(inputs["delta_t"]))

    def g(k):
        return np.asarray(inputs[k], np.float32)

    diff_w, diff_b = g("diff_w"), g("diff_b")
    tm_w1, tm_cb1 = g("tm_w1"), g("tm_cb1")
    tm_w2, tm_cb2 = g("tm_w2"), g("tm_cb2")

    pp = np.zeros((P, DO, NPARAM), np.float32)

    def put(i, v):
        pp[:, :, i] = v.reshape(DO, P).T

    put(C0, dt * diff_w[:, 0, 0])
    put(C1, dt * diff_w[:, 0, 1] + (1.0 - dt))
    put(C2, dt * diff_w[:, 0, 2])
    put(CB, dt * diff_b + g("lu_b2") + tm_cb2)
    put(T0, tm_w1[:, 0, 0])
    put(T1, tm_w1[:, 0, 1])
    put(T2, tm_w1[:, 0, 2])
    put(TCB1, tm_cb1)
    put(U0, tm_w2[:, 0, 0])
    put(U1, tm_w2[:, 0, 1])
    put(U2, tm_w2[:, 0, 2])
    put(TMG, g("tm_g"))
    put(TMB, g("tm_beta"))
    put(N1G, g("n1_g"))
    put(N1B, g("n1_b"))
    put(N2G, g("n2_g"))
    put(N2B, g("n2_b"))
    put(LUB1, g("lu_b1"))
    put(FFB1, g("ff_b1"))
    put(FFB2, g("ff_b2"))

    diags = np.zeros((P, 3, DO, P), np.float32)
    idx = np.arange(P)
    for tap in range(3):
        for dc in range(DO):
            diags[idx, tap, dc, idx] = tm_w2[dc * P + idx, 0, tap]
    diags = diags.astype(F16_NP)

    rows = np.zeros((1, 3 * D), np.float32)
    rows[0, 0:D] = g("bq") * WS
    rows[0, D:2 * D] = g("bk") * KS
    rows[0, 2 * D:3 * D] = g("bv") * WS
    rows = rows.astype(BF16_NP)

    wt = {}
    for name, key in (("w1T", "lu_w1"), ("w2T", "lu_w2"),
                      ("f1T", "ff_w1"), ("f2T", "ff_w2")):
        wt[name] = np.ascontiguousarray(g(key).T).astype(F16_NP)
    w8 = {}
    for name, key, s in (("wq8", "wq", WS), ("wk8", "wk", KS), ("wv8", "wv", WS)):
        w8[name] = np.ascontiguousarray(g(key).T * s).astype(F8_NP)

    xT = np.ascontiguousarray(x.transpose(0, 2, 1)).astype(F16_NP)
    x8 = xT.astype(F8_NP)

    flags = dict(
        use_bq=bool(np.any(g("bq"))),
        use_bk=bool(np.any(g("bk"))),
        use_bv=bool(np.any(g("bv"))),
        use_tmb=bool(np.any(g("tm_beta"))),
        use_n1b=bool(np.any(g("n1_b"))),
        use_n2b=bool(np.any(g("n2_b"))),
    )

    shared = {**wt, **w8, "pp": pp, "rows": rows, "diags": diags}
    in_maps = [{**shared, "x_T": xT[b], "x_8": x8[b]} for b in range(B)]
    return in_maps, flags, (B, N)


_NC_CACHE = {}


def kernel(**inputs):
    in_maps, flags, (B, N) = make_in_maps(inputs)
    key = (N, tuple(sorted(flags.items())))
    if key not in _NC_CACHE:
        _NC_CACHE[key] = build_nc(N=N, NT=512, **flags)
    nc = _NC_CACHE[key]
    res = run_bass_kernel_spmd(nc, in_maps, list(range(B)))
    y = np.stack([res.results[b]["y_T"] for b in range(B)])
    return np.ascontiguousarray(y.transpose(0, 2, 1)).astype(np.float32)


# revision 30
# speedup vs baseline: 1.0211x; 1.0211x over previous
"""Trainium2 Bass kernel for nn_DiffuRNNLayer (B=8, N=2048, D=1024).

Sharding: data-parallel over batch - one batch element per NeuronCore (8 cores).
Per-core phases:
  A: Q/K/V projections in fp8 DoubleRow (+elu+1); Qp (bf16, scaled by WS),
     Kp (fp8, x16) / V (fp8) SBUF-resident; K_sum accumulated inline.
  B: KV = Kp^T V via fp8 DoubleRow from SBUF-resident Kp/V.
  C: acc = dwconv''(x) + MLP(x) + tokenmixer(LN(x)) in fp16; spill acc.
  D: attn numerator with 1/norm folded into Qp, acc += attn; LN1; FFN
     residual; LN2; write y^T (fp16).
"""

import numpy as np
import ml_dtypes
from contextlib import ExitStack

import concourse.bass as bass
import concourse.bacc as bacc
import concourse.tile as tile
import concourse.mybir as mybir
from concourse.bass_utils import run_bass_kernel_spmd

F32 = mybir.dt.float32
BF16 = mybir.dt.bfloat16
F16 = mybir.dt.float16
F8 = mybir.dt.float8e4
AF = mybir.ActivationFunctionType
OP = mybir.AluOpType
DR = mybir.MatmulPerfMode.DoubleRow
BF16_NP = ml_dtypes.bfloat16
F16_NP = np.float16
F8_NP = ml_dtypes.float8_e4m3

P = 128
D = 1024
DO = D // P  # 8 chunks of the channel dim
WS = 256.0   # fp8 weight scale for wq/wv
KS = 16.0    # fp8 weight scale for wk (kp8 = KS*Kp must stay under 240)
LNWS = float(np.log(WS))
LNKS = float(np.log(KS))

# pp param-plane indices (per-partition params, laid out [128, DO, NP])
(C0, C1, C2, CB, T0, T1, T2, TCB1, U0, U1, U2,
 TMG, TMB, N1G, N1B, N2G, N2B, LUB1, FFB1, FFB2) = range(20)
NPARAM = 20


def build_nc(N=2048, NT=512, use_bq=False, use_bk=False, use_bv=False,
             use_tmb=False, use_n1b=False, use_n2b=False, use_fb2=False,
             debug=False):
    NTILES = N // NT
    NTA = 1024             # phase-A supertile width
    NST = N // NTA
    NCH_A = NTA // P       # 128-token chunks per supertile
    TOTCH = N // P
    W = NT + 4             # phase-C tile width with +-2 halo
    assert N % NT == 0 and NT % P == 0 and N % NTA == 0

    nc = bacc.Bacc(None, target_bir_lowering=False, debug=debug)

    xT_d = nc.dram_tensor("x_T", [D, N], F16, kind="ExternalInput")
    x8_d = nc.dram_tensor("x_8", [D, N], F8, kind="ExternalInput")
    w8_d = {}
    for name in ("wq8", "wk8", "wv8"):
        w8_d[name] = nc.dram_tensor(name, [D, D], F8, kind="ExternalInput")
    w_d = {}
    for name in ("w1T", "w2T"):
        w_d[name] = nc.dram_tensor(name, [D, D], F16, kind="ExternalInput")
    for name in ("f18", "f28"):
        w8_d[name] = nc.dram_tensor(name, [D, D], F8, kind="ExternalInput")
    pp_d = nc.dram_tensor("pp", [P, DO, NPARAM], F32, kind="ExternalInput")
    diags_d = nc.dram_tensor("diags", [P, 3, DO, P], F16, kind="ExternalInput")
    rows_d = nc.dram_tensor("rows", [1, 3 * D], BF16, kind="ExternalInput")
    yT_d = nc.dram_tensor("y_T", [D, N], F16, kind="ExternalOutput")

    acc_sp = nc.dram_tensor("acc_sp", [D, N], F16)

    xT = xT_d.rearrange("(o p) n -> p o n", p=P)
    x8r = x8_d.rearrange("(o p) n -> p o n", p=P)
    w8r = {k: v.rearrange("(o p) n -> p o n", p=P) for k, v in w8_d.items()}
    wr = {k: v.rearrange("(o p) n -> p o n", p=P) for k, v in w_d.items()}
    acc_r = acc_sp.rearrange("(o p) n -> p o n", p=P)
    yT = yT_d.rearrange("(o p) n -> p o n", p=P)

    with tile.TileContext(nc) as tc, ExitStack() as top:
        persist = top.enter_context(tc.tile_pool(name="persist", bufs=1))
        ones_1p_f32 = persist.tile([1, P], F32)
        nc.vector.memset(ones_1p_f32, 1.0)
        ones_one = persist.tile([1, 1], BF16)
        nc.vector.memset(ones_one, 1.0)
        ones8 = persist.tile([P, 2, 16], F8)
        nc.vector.memset(ones8, 1.0)
        ksrow_sb = persist.tile([1, D], BF16)
        onesD = persist.tile([P, P], F16)
        nc.vector.memset(onesD, 1.0 / D)
        eps_ln = persist.tile([P, 1], F32)
        nc.vector.memset(eps_ln, 1e-5)
        lnws = persist.tile([P, 1], F32)
        nc.vector.memset(lnws, LNWS)
        lnks = persist.tile([P, 1], F32)
        nc.vector.memset(lnks, LNKS)
        kv_sb = persist.tile([P, DO, D], BF16)
        ksum_sb = persist.tile([P, DO, 1], BF16)
        qp = persist.tile([P, DO, N], BF16)  # WS-scaled Qp, resident
        pp = persist.tile([P, DO, NPARAM], F32)
        diags = persist.tile([P, 3, DO, P], F16)
        rows = ones_row = ones_1p_bf = None
        if use_bq or use_bk or use_bv:
            rows = persist.tile([1, 3 * D], BF16)
            ones_row = persist.tile([1, NTA], BF16)
            nc.vector.memset(ones_row, 1.0)
            ones_1p_bf = persist.tile([1, P], BF16)
            nc.vector.memset(ones_1p_bf, 1.0)

        def stats_mm(psum, lhs_ones, rhs3, width):
            """Accumulate over DO k-chunks: psum[:, j] = mean over channel dim,
            replicated across partitions.  rhs3: [P, DO, width]."""
            for c0 in range(0, width, 512):
                cw = min(512, width - c0)
                for kc in range(DO):
                    nc.tensor.matmul(psum[:, c0:c0 + cw], lhs_ones,
                                     rhs3[:, kc, c0:c0 + cw],
                                     start=(kc == 0), stop=(kc == DO - 1))

        # ---------------- Phases A+B: QKV + KV (fp8 DoubleRow) ----------------
        with ExitStack() as ph:
            wpool = ph.enter_context(tc.tile_pool(name="wA", bufs=1))
            wq_sb = wpool.tile([P, DO, D], F8, tag="wq")
            nc.sync.dma_start(wq_sb, w8r["wq8"])
            kvres = ph.enter_context(tc.tile_pool(name="kvres", bufs=1))
            kp8 = kvres.tile([P, TOTCH, D], F8, tag="kp8")
            v8 = kvres.tile([P, TOTCH, D], F8, tag="v8")
            io = ph.enter_context(tc.tile_pool(name="ioA", bufs=2))
            x8_0 = io.tile([P, DO, NTA], F8, tag="x8A", name="x8_0")
            nc.sync.dma_start(x8_0, x8r[:, :, 0:NTA])
            wk_sb = wpool.tile([P, DO, D], F8, tag="wk")
            nc.sync.dma_start(wk_sb, w8r["wk8"])
            wv_sb = wpool.tile([P, DO, D], F8, tag="wv")
            nc.sync.dma_start(wv_sb, w8r["wv8"])
            if use_bq or use_bk or use_bv:
                nc.sync.dma_start(rows, rows_d[:])
            nc.sync.dma_start(pp, pp_d[:])
            nc.sync.dma_start(diags, diags_d[:])

            ev = ph.enter_context(tc.tile_pool(name="evA", bufs=4))
            pa = ph.enter_context(ExitStack())
            psQ = pa.enter_context(tc.tile_pool(name="psQA", bufs=1, space="PSUM"))
            psK = pa.enter_context(tc.tile_pool(name="psKA", bufs=2, space="PSUM"))
            psV = pa.enter_context(tc.tile_pool(name="psVA", bufs=2, space="PSUM"))
            psKS = pa.enter_context(tc.tile_pool(name="psKSA", bufs=1, space="PSUM"))
            ps_ks = [psKS.tile([1, 512], F32, tag=f"ksr{h}", name=f"ksr{h}")
                     for h in range(2)]

            def ksum_pair(chp):
                c2 = slice(2 * chp, 2 * chp + 2)
                for h in range(2):
                    nc.tensor.matmul(ps_ks[h], ones8[:, :, 0:1],
                                     kp8[:, c2, h * 512:(h + 1) * 512],
                                     start=(chp == 0), stop=(chp == TOTCH // 2 - 1),
                                     perf_mode=DR)

            def do_q(st, x8_t, n0):
                for dc in range(DO):
                    ps_q = psQ.tile([P, NTA], F32, tag="psq", name=f"psq{st}_{dc}")
                    for kcp in range(DO // 2):
                        ks2 = slice(2 * kcp, 2 * kcp + 2)
                        for h in range(2):
                            hs = slice(h * 512, (h + 1) * 512)
                            nc.tensor.matmul(ps_q[:, hs],
                                             wq_sb[:, ks2, dc * P:(dc + 1) * P],
                                             x8_t[:, ks2, hs],
                                             start=(kcp == 0),
                                             stop=(kcp == DO // 2 - 1 and not use_bq),
                                             perf_mode=DR)
                    if use_bq:
                        for h in range(2):
                            hs = slice(h * 512, (h + 1) * 512)
                            nc.tensor.matmul(ps_q[:, hs],
                                             rows[0:1, dc * P:(dc + 1) * P],
                                             ones_row[0:1, hs], start=False,
                                             stop=True)
                    for h in range(2):
                        hs = slice(h * 512, (h + 1) * 512)
                        # e_all = WS*exp(q); e1 = min(e_all, WS) = WS*exp(min(q,0))
                        e_all = ev.tile([P, 512], BF16, tag="eQ")
                        nc.scalar.activation(e_all, ps_q[:, hs], AF.Exp,
                                             scale=1.0 / WS, bias=lnws[:, 0:1])
                        e1 = ev.tile([P, 512], BF16, tag="e1Q")
                        nc.vector.tensor_scalar_min(e1, e_all, float(WS))
                        # qp' = max(WS*q, 0) + WS*exp(min(q,0)) = WS*Qp
                        nc.vector.scalar_tensor_tensor(
                            qp[:, dc, n0 + h * 512:n0 + (h + 1) * 512],
                            ps_q[:, hs], 0.0, e1, OP.max, OP.add)

            def do_kv(st, x8_t, n0):
                for ch in range(NCH_A):
                    cs = slice(ch * P, (ch + 1) * P)
                    chg = st * NCH_A + ch
                    for h in range(2):
                        hs = slice(h * 512, (h + 1) * 512)
                        ps_k = psK.tile([P, 512], F32, tag="psk")
                        ps_v = psV.tile([P, 512], F32, tag="psv")
                        for kcp in range(DO // 2):
                            ks2 = slice(2 * kcp, 2 * kcp + 2)
                            nc.tensor.matmul(ps_k, x8_t[:, ks2, cs],
                                             wk_sb[:, ks2, hs],
                                             start=(kcp == 0),
                                             stop=(kcp == DO // 2 - 1 and not use_bk),
                                             perf_mode=DR)
                            nc.tensor.matmul(ps_v, x8_t[:, ks2, cs],
                                             wv_sb[:, ks2, hs],
                                             start=(kcp == 0),
                                             stop=(kcp == DO // 2 - 1 and not use_bv),
                                             perf_mode=DR)
                        if use_bk:
                            nc.tensor.matmul(ps_k, ones_1p_bf[0:1, :],
                                             rows[0:1, D + h * 512:D + (h + 1) * 512],
                                             start=False, stop=True)
                        if use_bv:
                            nc.tensor.matmul(ps_v, ones_1p_bf[0:1, :],
                                             rows[0:1, 2 * D + h * 512:2 * D + (h + 1) * 512],
                                             start=False, stop=True)
                        # kp8 = KS*Kp = max(KS*k, 0) + min(KS*e^k, KS)
                        ek = ev.tile([P, 512], BF16, tag="eK")
                        nc.scalar.activation(ek, ps_k, AF.Exp,
                                             scale=1.0 / KS, bias=lnks[:, 0:1])
                        e1k = ev.tile([P, 512], BF16, tag="e1K")
                        nc.vector.tensor_scalar_min(e1k, ek, float(KS))
                        nc.vector.scalar_tensor_tensor(kp8[:, chg, hs], ps_k, 0.0,
                                                       e1k, OP.max, OP.add)
                        nc.scalar.activation(v8[:, chg, hs], ps_v, AF.Copy,
                                             scale=1.0 / WS)
                    # K_sum pairs lag ~2 pairs behind so the DVE readout
                    # chain has drained (pair p needs kp8 chunks 2p, 2p+1)
                    if chg % 2 == 1 and chg >= 5:
                        ksum_pair((chg - 5) // 2)

            for st in range(NST):
                n0 = st * NTA
                if st == 0:
                    x8_t = x8_0
                else:
                    x8_t = io.tile([P, DO, NTA], F8, tag="x8A", name=f"x8_{st}")
                    nc.sync.dma_start(x8_t, x8r[:, :, n0:n0 + NTA])
                if st < NST - 1:
                    do_q(st, x8_t, n0)
                    do_kv(st, x8_t, n0)
                else:
                    # last supertile: K/V first so phase B's inputs finish early
                    do_kv(st, x8_t, n0)
                    do_q(st, x8_t, n0)

            # drain remaining K_sum pairs (pairs 0..(TOTCH-6)//2 issued inline)
            for chp in range((TOTCH - 4) // 2, TOTCH // 2):
                ksum_pair(chp)
            for h in range(2):
                hs = slice(h * 512, (h + 1) * 512)
                nc.scalar.activation(ksrow_sb[0:1, hs], ps_ks[h], AF.Copy,
                                     scale=1.0 / KS)
            pa.close()

            # ---------------- Phase B: KV accumulation (fp8 DR) ----------------
            psB = ph.enter_context(tc.tile_pool(name="psB", bufs=1, space="PSUM"))
            for pass_ in range(4):
                kv_ps = [psB.tile([P, NTA], F32, tag=f"kvps{i}", name=f"kvps{pass_}_{i}")
                         for i in range(2)]
                for chp in range(TOTCH // 2):
                    c2 = slice(2 * chp, 2 * chp + 2)
                    for i in range(2):
                        dc = pass_ * 2 + i
                        for h in range(2):
                            hs = slice(h * 512, (h + 1) * 512)
                            nc.tensor.matmul(kv_ps[i][:, hs],
                                             kp8[:, c2, dc * P:(dc + 1) * P],
                                             v8[:, c2, hs],
                                             start=(chp == 0),
                                             stop=(chp == TOTCH // 2 - 1),
                                             perf_mode=DR)
                for i in range(2):
                    nc.scalar.activation(kv_sb[:, pass_ * 2 + i, :], kv_ps[i],
                                         AF.Copy, scale=1.0 / KS)

            # transpose K_sum row -> per-partition column layout [P, DO]
            psks = ph.enter_context(tc.tile_pool(name="psks", bufs=1, space="PSUM"))
            ps_ksc = psks.tile([P, DO], F32, tag="kscol")
            for dc in range(DO):
                nc.tensor.matmul(ps_ksc[:, dc:dc + 1],
                                 ksrow_sb[0:1, dc * P:(dc + 1) * P],
                                 ones_one[0:1, 0:1], start=True, stop=True)
            nc.scalar.activation(ksum_sb[:, :, 0], ps_ksc, AF.Copy)

        # ---------------- Phase C: conv'' + local MLP + token mixer ----------------
        with ExitStack() as ph:
            wpool = ph.enter_context(tc.tile_pool(name="wC", bufs=1))
            w1_sb = wpool.tile([P, DO, D], F16, tag="w1")
            nc.sync.dma_start(w1_sb, wr["w1T"])
            w2_sb = wpool.tile([P, DO, D], F16, tag="w2")
            nc.sync.dma_start(w2_sb, wr["w2T"])
            io = ph.enter_context(tc.tile_pool(name="ioC", bufs=2))
            pipe = ph.enter_context(tc.tile_pool(name="pipeC", bufs=2))
            mid = ph.enter_context(tc.tile_pool(name="midC", bufs=1))
            sm = ph.enter_context(tc.tile_pool(name="smC", bufs=1))
            ps = ph.enter_context(tc.tile_pool(name="psC", bufs=2, space="PSUM"))
            pst = ph.enter_context(tc.tile_pool(name="pstC", bufs=1, space="PSUM"))

            def c_front(it):
                n0 = it * NT
                x_t = io.tile([P, DO, W], F16, tag="xC", name=f"x_{it}")
                lo, hi = n0 - 2, n0 + NT + 2
                if lo < 0:
                    nc.vector.memset(x_t[:, :, 0:2], 0.0)
                    nc.sync.dma_start(x_t[:, :, 2:W], xT[:, :, 0:hi])
                elif hi > N:
                    nc.vector.memset(x_t[:, :, W - 2:W], 0.0)
                    nc.sync.dma_start(x_t[:, :, 0:W - 2], xT[:, :, lo:N])
                else:
                    nc.sync.dma_start(x_t, xT[:, :, lo:hi])

                dcv = io.tile([P, DO, NT], F16, tag="dcvC", name=f"dcv_{it}")
                # diffusion dwconv'': center tap on ACT, side taps on DVE
                for o in range(DO):
                    nc.scalar.activation(dcv[:, o, :], x_t[:, o, 2:NT + 2],
                                         AF.Identity, bias=pp[:, o, CB:CB + 1],
                                         scale=pp[:, o, C1:C1 + 1])
                for o in range(DO):
                    nc.vector.scalar_tensor_tensor(dcv[:, o, :], x_t[:, o, 1:NT + 1],
                                                   pp[:, o, C0:C0 + 1], dcv[:, o, :],
                                                   OP.mult, OP.add)
                for o in range(DO):
                    nc.vector.scalar_tensor_tensor(dcv[:, o, :], x_t[:, o, 3:NT + 3],
                                                   pp[:, o, C2:C2 + 1], dcv[:, o, :],
                                                   OP.mult, OP.add)

                # local MLP first half
                h1_t = pipe.tile([P, DO, NT], F16, tag="h1", name=f"h1_{it}")
                for dc in range(DO):
                    ps_h = ps.tile([P, NT], F32, tag="psh1", name=f"psh1_{it}_{dc}")
                    for kc in range(DO):
                        nc.tensor.matmul(ps_h, w1_sb[:, kc, dc * P:(dc + 1) * P],
                                         x_t[:, kc, 2:NT + 2],
                                         start=(kc == 0), stop=(kc == DO - 1))
                    nc.scalar.activation(h1_t[:, dc, :], ps_h, AF.Gelu,
                                         bias=pp[:, dc, LUB1:LUB1 + 1])

                # token mixer LN stats
                sq_t = mid.tile([P, DO, W], F16, tag="tokA", name=f"sq_{it}")
                nc.scalar.activation(sq_t, x_t, AF.Square)
                ps_m = pst.tile([P, W], F32, tag="psm", name=f"psm_{it}")
                stats_mm(ps_m, onesD, x_t, W)
                ps_s = pst.tile([P, W], F32, tag="pss", name=f"pss_{it}")
                stats_mm(ps_s, onesD, sq_t, W)
                m_sb = sm.tile([P, W], F16, tag="msb", name=f"msb_{it}")
                nc.scalar.activation(m_sb, ps_m, AF.Copy)
                var = sm.tile([P, W], F32, tag="var", name=f"var_{it}")
                nc.scalar.activation(var, ps_m, AF.Square)
                nc.vector.tensor_sub(var, ps_s, var)
                nc.scalar.activation(var, var, AF.Sqrt, bias=eps_ln[:, 0:1])
                nc.vector.reciprocal_approx_fast(out=var, in_=var)
                rstd = sm.tile([P, W], F16, tag="rstd", name=f"rstd_{it}")
                nc.vector.tensor_copy(rstd, var)
                u_t = mid.tile([P, DO, W], F16, tag="tokA", name=f"u_{it}")
                for o in range(DO):
                    nc.vector.tensor_sub(u_t[:, o, :], x_t[:, o, :], m_sb)
                xm_t = mid.tile([P, DO, W], F16, tag="tokC", name=f"xm_{it}")
                for o in range(DO):
                    nc.vector.scalar_tensor_tensor(xm_t[:, o, :], u_t[:, o, :],
                                                   pp[:, o, TMG:TMG + 1], rstd,
                                                   OP.mult, OP.mult)
                if use_tmb:
                    for o in range(DO):
                        nc.vector.tensor_scalar_add(xm_t[:, o, :], xm_t[:, o, :],
                                                    pp[:, o, TMB:TMB + 1])
                # conv1: t_s[k] = conv1(xm)[k+1], k in [0, W-2)
                t_t = mid.tile([P, DO, W - 2], F16, tag="tokD", name=f"t_{it}")
                for o in range(DO):
                    nc.scalar.activation(t_t[:, o, :], xm_t[:, o, 1:W - 1],
                                         AF.Identity, bias=pp[:, o, TCB1:TCB1 + 1],
                                         scale=pp[:, o, T1:T1 + 1])
                for o in range(DO):
                    nc.vector.scalar_tensor_tensor(t_t[:, o, :], xm_t[:, o, 0:W - 2],
                                                   pp[:, o, T0:T0 + 1],
                                                   t_t[:, o, :], OP.mult, OP.add)
                for o in range(DO):
                    nc.vector.scalar_tensor_tensor(t_t[:, o, :], xm_t[:, o, 2:W],
                                                   pp[:, o, T2:T2 + 1],
                                                   t_t[:, o, :], OP.mult, OP.add)
                t2_t = pipe.tile([P, DO, W - 2], F16, tag="t2", name=f"t2_{it}")
                nc.scalar.activation(t2_t, t_t, AF.Gelu)
                if it == 0:
                    nc.vector.memset(t2_t[:, :, 0:1], 0.0)
                if it == NTILES - 1:
                    nc.vector.memset(t2_t[:, :, W - 3:W - 2], 0.0)
                return x_t, dcv, h1_t, t2_t

            def c_back(it, tiles):
                n0 = it * NT
                x_t, dcv, h1_t, t2_t = tiles
                acc = io.tile([P, DO, NT], F16, tag="accC", name=f"acc_{it}")
                for dc in range(DO):
                    ps_h = ps.tile([P, NT], F32, tag="psh2", name=f"psh2_{it}_{dc}")
                    for kc in range(DO):
                        nc.tensor.matmul(ps_h, w2_sb[:, kc, dc * P:(dc + 1) * P],
                                         h1_t[:, kc, :],
                                         start=(kc == 0), stop=False)
                    for tap in range(3):
                        nc.tensor.matmul(ps_h, diags[:, tap, dc, :],
                                         t2_t[:, dc, tap:NT + tap],
                                         start=False, stop=(tap == 2))
                    nc.vector.tensor_add(acc[:, dc, :], ps_h, dcv[:, dc, :])
                nc.sync.dma_start(acc_r[:, :, n0:n0 + NT], acc)

            pend = {0: c_front(0)}
            for it in range(NTILES):
                if it + 1 < NTILES:
                    pend[it + 1] = c_front(it + 1)
                c_back(it, pend.pop(it))

        # ---------------- Phase D: attention + LN1 + FFN + LN2 ----------------
        # Pipelined; numerator of tile t+1 is split into two half-groups that
        # are issued under tile t's two DVE-bound LN chains.
        with ExitStack() as ph:
            wpoolD = ph.enter_context(tc.tile_pool(name="wD", bufs=1))
            f1_sb = wpoolD.tile([P, DO, D], F8, tag="f1")
            nc.sync.dma_start(f1_sb, w8r["f18"])
            f2_sb = wpoolD.tile([P, DO, D], F8, tag="f2")
            nc.sync.dma_start(f2_sb, w8r["f28"])
            io = ph.enter_context(tc.tile_pool(name="ioD", bufs=3))
            mid = ph.enter_context(tc.tile_pool(name="midD", bufs=1))
            sm = ph.enter_context(tc.tile_pool(name="smD", bufs=2))
            ps = ph.enter_context(tc.tile_pool(name="psD", bufs=2, space="PSUM"))
            psf_pool = ph.enter_context(tc.tile_pool(name="psfD", bufs=2, space="PSUM"))
            pst = ph.enter_context(tc.tile_pool(name="pstD", bufs=1, space="PSUM"))

            def d_load(it):
                n0 = it * NT
                acc_t = io.tile([P, DO, NT], F16, tag="accD", name=f"accD_{it}")
                nc.sync.dma_start(acc_t, acc_r[:, :, n0:n0 + NT])
                return acc_t

            def d_front_a(it, acc_t):
                """norm row, 1/norm fold, numerator halves 0-3."""
                n0 = it * NT
                ps_n = ps.tile([P, NT], F32, tag="psnum", name=f"psn_{it}")
                for kc in range(DO):
                    nc.tensor.matmul(ps_n[0:1, :], ksum_sb[:, kc, :],
                                     qp[:, kc, n0:n0 + NT],
                                     start=(kc == 0), stop=(kc == DO - 1))
                nr = sm.tile([1, NT], F32, tag="nrD", name=f"nr_{it}")
                nc.vector.tensor_scalar_add(nr, ps_n[0:1, :], 1e-6)
                rr = sm.tile([1, NT], F32, tag="rrD", name=f"rr_{it}")
                nc.vector.reciprocal_approx_fast(out=rr, in_=nr)
                ps_rep = ps.tile([P, NT], F32, tag="psnum", name=f"psrep_{it}")
                nc.tensor.matmul(ps_rep, ones_1p_f32[0:1, :], rr, start=True,
                                 stop=True)
                rep_sb = mid.tile([P, NT], BF16, tag="repsb", name=f"rep_{it}")
                nc.scalar.activation(rep_sb, ps_rep, AF.Copy)
                for kc in range(DO):
                    nc.vector.tensor_mul(qp[:, kc, n0:n0 + NT],
                                         qp[:, kc, n0:n0 + NT], rep_sb)
                for ec in range(DO // 2):
                    ps_u = ps.tile([P, NT], F32, tag="psnum", name=f"psnum_{it}_{ec}")
                    for kc in range(DO):
                        nc.tensor.matmul(ps_u, kv_sb[:, kc, ec * P:(ec + 1) * P],
                                         qp[:, kc, n0:n0 + NT],
                                         start=(kc == 0), stop=(kc == DO - 1))
                    nc.vector.tensor_add(acc_t[:, ec, :], acc_t[:, ec, :], ps_u)
                return acc_t

            def d_front_b(it, acc_t):
                n0 = it * NT
                for ec in range(DO // 2, DO):
                    ps_u = ps.tile([P, NT], F32, tag="psnum", name=f"psnum_{it}_{ec}")
                    for kc in range(DO):
                        nc.tensor.matmul(ps_u, kv_sb[:, kc, ec * P:(ec + 1) * P],
                                         qp[:, kc, n0:n0 + NT],
                                         start=(kc == 0), stop=(kc == DO - 1))
                    nc.vector.tensor_add(acc_t[:, ec, :], acc_t[:, ec, :], ps_u)
                return acc_t

            def d_mid(it, acc_t):
                """LN1 stats + apply -> y1 (fp16)."""
                sq_t = mid.tile([P, DO, NT], F16, tag="sqD", name=f"sqD_{it}")
                nc.scalar.activation(sq_t, acc_t, AF.Square)
                ps_m1 = pst.tile([P, NT], F32, tag="psm1", name=f"psm1_{it}")
                stats_mm(ps_m1, onesD, acc_t, NT)
                ps_s1 = pst.tile([P, NT], F32, tag="pss1", name=f"pss1_{it}")
                stats_mm(ps_s1, onesD, sq_t, NT)
                m1_sb = sm.tile([P, NT], F16, tag="m1sb", name=f"m1_{it}")
                nc.scalar.activation(m1_sb, ps_m1, AF.Copy)
                var1 = sm.tile([P, NT], F32, tag="varD", name=f"var1_{it}")
                nc.scalar.activation(var1, ps_m1, AF.Square)
                nc.vector.tensor_sub(var1, ps_s1, var1)
                nc.scalar.activation(var1, var1, AF.Sqrt, bias=eps_ln[:, 0:1])
                nc.vector.reciprocal_approx_fast(out=var1, in_=var1)
                rstd1 = sm.tile([P, NT], F16, tag="rstdb", name=f"rstdb_{it}")
                nc.vector.tensor_copy(rstd1, var1)
                u1_t = mid.tile([P, DO, NT], F16, tag="u1", name=f"u1_{it}")
                for o in range(DO):
                    nc.vector.tensor_sub(u1_t[:, o, :], acc_t[:, o, :], m1_sb)
                y1_t = mid.tile([P, DO, NT], F16, tag="y1", name=f"y1_{it}")
                for o in range(DO):
                    nc.vector.scalar_tensor_tensor(y1_t[:, o, :], u1_t[:, o, :],
                                                   pp[:, o, N1G:N1G + 1], rstd1,
                                                   OP.mult, OP.mult)
                if use_n1b:
                    for o in range(DO):
                        nc.vector.tensor_scalar_add(y1_t[:, o, :], y1_t[:, o, :],
                                                    pp[:, o, N1B:N1B + 1])
                return y1_t

            def d_ffn(it, y1_t):
                # fp8 cast of y1 for the DR matmuls (residual stays fp16)
                y18_t = mid.tile([P, DO, NT], F8, tag="y18", name=f"y18_{it}")
                nc.scalar.activation(y18_t, y1_t, AF.Copy)
                f1h_t = mid.tile([P, DO, NT], F8, tag="f1h", name=f"f1h_{it}")
                for dc in range(DO):
                    ps_f = psf_pool.tile([P, NT], F32, tag="psf",
                                         name=f"psf1_{it}_{dc}")
                    for kcp in range(DO // 2):
                        ks2 = slice(2 * kcp, 2 * kcp + 2)
                        nc.tensor.matmul(ps_f, f1_sb[:, ks2, dc * P:(dc + 1) * P],
                                         y18_t[:, ks2, :],
                                         start=(kcp == 0), stop=(kcp == DO // 2 - 1),
                                         perf_mode=DR)
                    nc.scalar.activation(f1h_t[:, dc, :], ps_f, AF.Gelu,
                                         scale=1.0 / WS,
                                         bias=pp[:, dc, FFB1:FFB1 + 1])
                y2_t = mid.tile([P, DO, NT], F16, tag="y2", name=f"y2_{it}")
                for dc in range(DO):
                    ps_f = psf_pool.tile([P, NT], F32, tag="psf",
                                         name=f"psf2_{it}_{dc}")
                    for kcp in range(DO // 2):
                        ks2 = slice(2 * kcp, 2 * kcp + 2)
                        nc.tensor.matmul(ps_f, f2_sb[:, ks2, dc * P:(dc + 1) * P],
                                         f1h_t[:, ks2, :],
                                         start=(kcp == 0), stop=(kcp == DO // 2 - 1),
                                         perf_mode=DR)
                    nc.vector.scalar_tensor_tensor(y2_t[:, dc, :], ps_f,
                                                   1.0 / WS,
                                                   y1_t[:, dc, :], OP.mult, OP.add)
                    if use_fb2:
                        nc.scalar.activation(y2_t[:, dc, :], y2_t[:, dc, :],
                                             AF.Identity,
                                             bias=pp[:, dc, FFB2:FFB2 + 1])
                return y2_t

            def d_back(it, y2_t):
                n0 = it * NT
                sq2_t = mid.tile([P, DO, NT], F16, tag="sqD", name=f"sq2_{it}")
                nc.scalar.activation(sq2_t, y2_t, AF.Square)
                ps_m2 = pst.tile([P, NT], F32, tag="psm2", name=f"psm2_{it}")
                stats_mm(ps_m2, onesD, y2_t, NT)
                ps_s2 = pst.tile([P, NT], F32, tag="pss2", name=f"pss2_{it}")
                stats_mm(ps_s2, onesD, sq2_t, NT)
                m2_sb = sm.tile([P, NT], F32, tag="m2sb", name=f"m2_{it}")
                nc.scalar.activation(m2_sb, ps_m2, AF.Copy)
                var2 = sm.tile([P, NT], F32, tag="varD", name=f"var2_{it}")
                nc.scalar.activation(var2, ps_m2, AF.Square)
                nc.vector.tensor_sub(var2, ps_s2, var2)
                nc.scalar.activation(var2, var2, AF.Sqrt, bias=eps_ln[:, 0:1])
                nc.vector.reciprocal_approx_fast(out=var2, in_=var2)
                yo_t = mid.tile([P, DO, NT], F16, tag="yo", name=f"yo_{it}")
                for o in range(DO):
                    nc.vector.tensor_sub(yo_t[:, o, :], y2_t[:, o, :], m2_sb)
                for o in range(DO):
                    nc.vector.scalar_tensor_tensor(yo_t[:, o, :], yo_t[:, o, :],
                                                   pp[:, o, N2G:N2G + 1], var2,
                                                   OP.mult, OP.mult)
                if use_n2b:
                    for o in range(DO):
                        nc.vector.tensor_scalar_add(yo_t[:, o, :], yo_t[:, o, :],
                                                    pp[:, o, N2B:N2B + 1])
                nc.sync.dma_start(yT[:, :, n0:n0 + NT], yo_t)

            acc_tiles = {0: d_load(0)}
            if NTILES > 1:
                acc_tiles[1] = d_load(1)
            acc_cur = d_front_b(0, d_front_a(0, acc_tiles.pop(0)))
            acc_nxt = None
            for it in range(NTILES):
                y1_cur = d_mid(it, acc_cur)
                if it + 2 < NTILES:
                    acc_tiles[it + 2] = d_load(it + 2)
                if it + 1 < NTILES:
                    acc_nxt = d_front_a(it + 1, acc_tiles.pop(it + 1))
                y2_cur = d_ffn(it, y1_cur)
                if it + 1 < NTILES:
                    acc_cur = d_front_b(it + 1, acc_nxt)
                d_back(it, y2_cur)

    nc.compile()
    return nc


def make_in_maps(inputs, n_cores=8):
    """Host-side preprocessing: fold constants, transpose, cast, shard."""
    x = np.asarray(inputs["x"], np.float32)
    B, N, D_ = x.shape
    dt = float(np.asarray(inputs["delta_t"]))

    def g(k):
        return np.asarray(inputs[k], np.float32)

    diff_w, diff_b = g("diff_w"), g("diff_b")
    tm_w1, tm_cb1 = g("tm_w1"), g("tm_cb1")
    tm_w2, tm_cb2 = g("tm_w2"), g("tm_cb2")

    pp = np.zeros((P, DO, NPARAM), np.float32)

    def put(i, v):
        pp[:, :, i] = v.reshape(DO, P).T

    put(C0, dt * diff_w[:, 0, 0])
    put(C1, dt * diff_w[:, 0, 1] + (1.0 - dt))
    put(C2, dt * diff_w[:, 0, 2])
    put(CB, dt * diff_b + g("lu_b2") + tm_cb2)
    put(T0, tm_w1[:, 0, 0])
    put(T1, tm_w1[:, 0, 1])
    put(T2, tm_w1[:, 0, 2])
    put(TCB1, tm_cb1)
    put(U0, tm_w2[:, 0, 0])
    put(U1, tm_w2[:, 0, 1])
    put(U2, tm_w2[:, 0, 2])
    put(TMG, g("tm_g"))
    put(TMB, g("tm_beta"))
    put(N1G, g("n1_g"))
    put(N1B, g("n1_b"))
    put(N2G, g("n2_g"))
    put(N2B, g("n2_b"))
    put(LUB1, g("lu_b1"))
    put(FFB1, g("ff_b1"))
    put(FFB2, g("ff_b2"))

    diags = np.zeros((P, 3, DO, P), np.float32)
    idx = np.arange(P)
    for tap in range(3):
        for dc in range(DO):
            diags[idx, tap, dc, idx] = tm_w2[dc * P + idx, 0, tap]
    diags = diags.astype(F16_NP)

    rows = np.zeros((1, 3 * D), np.float32)
    rows[0, 0:D] = g("bq") * WS
    rows[0, D:2 * D] = g("bk") * KS
    rows[0, 2 * D:3 * D] = g("bv") * WS
    rows = rows.astype(BF16_NP)

    wt = {}
    for name, key in (("w1T", "lu_w1"), ("w2T", "lu_w2")):
        wt[name] = np.ascontiguousarray(g(key).T).astype(F16_NP)
    w8 = {}
    for name, key, s in (("wq8", "wq", WS), ("wk8", "wk", KS), ("wv8", "wv", WS),
                         ("f18", "ff_w1", WS), ("f28", "ff_w2", WS)):
        w8[name] = np.ascontiguousarray(g(key).T * s).astype(F8_NP)

    xT = np.ascontiguousarray(x.transpose(0, 2, 1)).astype(F16_NP)
    x8 = xT.astype(F8_NP)

    flags = dict(
        use_bq=bool(np.any(g("bq"))),
        use_bk=bool(np.any(g("bk"))),
        use_bv=bool(np.any(g("bv"))),
        use_tmb=bool(np.any(g("tm_beta"))),
        use_n1b=bool(np.any(g("n1_b"))),
        use_n2b=bool(np.any(g("n2_b"))),
        use_fb2=bool(np.any(g("ff_b2"))),
    )

    shared = {**wt, **w8, "pp": pp, "rows": rows, "diags": diags}
    in_maps = [{**shared, "x_T": xT[b], "x_8": x8[b]} for b in range(B)]
    return in_maps, flags, (B, N)


_NC_CACHE = {}


def kernel(**inputs):
    in_maps, flags, (B, N) = make_in_maps(inputs)
    key = (N, tuple(sorted(flags.items())))
    if key not in _NC_CACHE:
        _NC_CACHE[key] = build_nc(N=N, NT=512, **flags)
    nc = _NC_CACHE[key]
    res = run_bass_kernel_spmd(nc, in_maps, list(range(B)))
    y = np.stack([res.results[b]["y_T"] for b in range(B)])
    return np.ascontiguousarray(y.transpose(0, 2, 1)).astype(np.float32)


# revision 44
# speedup vs baseline: 1.0912x; 1.0686x over previous
"""Trainium2 Bass kernel for nn_DiffuRNNLayer (B=8, N=2048, D=1024).

Sharding: data-parallel over batch - one batch element per NeuronCore (8 cores).
Per-core phases:
  A: Q/K/V projections in fp8 DoubleRow (+elu+1); Qp (bf16, scaled by WS),
     Kp (fp8, x16) / V (fp8) SBUF-resident; K_sum accumulated inline.
  B: KV = Kp^T V via fp8 DoubleRow from SBUF-resident Kp/V.
  C: acc = dwconv''(x) + MLP(x) + tokenmixer(LN(x)) in fp16; spill acc.
  D: attn numerator with 1/norm folded into Qp, acc += attn; LN1; FFN
     residual; LN2; write y^T (fp16).
"""

import numpy as np
import ml_dtypes
from contextlib import ExitStack

import concourse.bass as bass
import concourse.bacc as bacc
import concourse.tile as tile
import concourse.mybir as mybir
from concourse.bass_utils import run_bass_kernel_spmd

F32 = mybir.dt.float32
BF16 = mybir.dt.bfloat16
F16 = mybir.dt.float16
F8 = mybir.dt.float8e4
AF = mybir.ActivationFunctionType
OP = mybir.AluOpType
DR = mybir.MatmulPerfMode.DoubleRow
BF16_NP = ml_dtypes.bfloat16
F16_NP = np.float16
F8_NP = ml_dtypes.float8_e4m3

P = 128
D = 1024
DO = D // P  # 8 chunks of the channel dim
WS = 256.0   # fp8 weight scale for wq/wv
KS = 16.0    # fp8 weight scale for wk (kp8 = KS*Kp must stay under 240)
LNWS = float(np.log(WS))
LNKS = float(np.log(KS))

# pp param-plane indices (per-partition params, laid out [128, DO, NP])
(C0, C1, C2, CB, T0, T1, T2, TCB1, U0, U1, U2,
 TMG, TMB, N1G, N1B, N2G, N2B, LUB1, FFB1, FFB2) = range(20)
NPARAM = 20


def build_nc(N=2048, NT=512, use_bq=False, use_bk=False, use_bv=False,
             use_tmb=False, use_n1b=False, use_n2b=False, use_fb2=False,
             debug=False):
    NTILES = N // NT
    NTA = 1024             # phase-A supertile width
    NST = N // NTA
    NCH_A = NTA // P       # 128-token chunks per supertile
    TOTCH = N // P
    W = NT + 4             # phase-C tile width with +-2 halo
    assert N % NT == 0 and NT % P == 0 and N % NTA == 0

    nc = bacc.Bacc(None, target_bir_lowering=False, debug=debug)

    xT_d = nc.dram_tensor("x_T", [D, N], F16, kind="ExternalInput")
    x8_d = nc.dram_tensor("x_8", [D, N], F8, kind="ExternalInput")
    w8_d = {}
    for name in ("wq8", "wk8", "wv8"):
        w8_d[name] = nc.dram_tensor(name, [D, D], F8, kind="ExternalInput")
    w_d = {}
    for name in ("w1T", "w2T"):
        w_d[name] = nc.dram_tensor(name, [D, D], F16, kind="ExternalInput")
    for name in ("f18", "f28"):
        w8_d[name] = nc.dram_tensor(name, [D, D], F8, kind="ExternalInput")
    pp_d = nc.dram_tensor("pp", [P, DO, NPARAM], F32, kind="ExternalInput")
    diags_d = nc.dram_tensor("diags", [P, 3, DO, P], F16, kind="ExternalInput")
    rows_d = nc.dram_tensor("rows", [1, 3 * D], BF16, kind="ExternalInput")
    yT_d = nc.dram_tensor("y_T", [D, N], F16, kind="ExternalOutput")

    acc_sp = nc.dram_tensor("acc_sp", [D, N], F16)

    xT = xT_d.rearrange("(o p) n -> p o n", p=P)
    x8r = x8_d.rearrange("(o p) n -> p o n", p=P)
    w8r = {k: v.rearrange("(o p) n -> p o n", p=P) for k, v in w8_d.items()}
    wr = {k: v.rearrange("(o p) n -> p o n", p=P) for k, v in w_d.items()}
    acc_r = acc_sp.rearrange("(o p) n -> p o n", p=P)
    yT = yT_d.rearrange("(o p) n -> p o n", p=P)

    with tile.TileContext(nc) as tc, ExitStack() as top:
        persist = top.enter_context(tc.tile_pool(name="persist", bufs=1))
        ones_1p_f32 = persist.tile([1, P], F32)
        nc.vector.memset(ones_1p_f32, 1.0)
        ones_one = persist.tile([1, 1], BF16)
        nc.vector.memset(ones_one, 1.0)
        ones8 = persist.tile([P, 2, 16], F8)
        nc.vector.memset(ones8, 1.0)
        ksrow_sb = persist.tile([1, D], BF16)
        onesD = persist.tile([P, P], F16)
        nc.vector.memset(onesD, 1.0 / D)
        eps_ln = persist.tile([P, 1], F32)
        nc.vector.memset(eps_ln, 1e-5)
        lnws = persist.tile([P, 1], F32)
        nc.vector.memset(lnws, LNWS)
        lnks = persist.tile([P, 1], F32)
        nc.vector.memset(lnks, LNKS)
        kv_sb = persist.tile([P, DO, D], BF16)
        ksum_sb = persist.tile([P, DO, 1], BF16)
        qp = persist.tile([P, DO, N], BF16)  # WS-scaled Qp, resident
        pp = persist.tile([P, DO, NPARAM], F32)
        diags = persist.tile([P, 3, DO, P], F16)
        rows = ones_row = ones_1p_bf = None
        if use_bq or use_bk or use_bv:
            rows = persist.tile([1, 3 * D], BF16)
            ones_row = persist.tile([1, NTA], BF16)
            nc.vector.memset(ones_row, 1.0)
            ones_1p_bf = persist.tile([1, P], BF16)
            nc.vector.memset(ones_1p_bf, 1.0)

        def stats_mm(psum, lhs_ones, rhs3, width):
            """Accumulate over DO k-chunks: psum[:, j] = mean over channel dim,
            replicated across partitions.  rhs3: [P, DO, width]."""
            for c0 in range(0, width, 512):
                cw = min(512, width - c0)
                for kc in range(DO):
                    nc.tensor.matmul(psum[:, c0:c0 + cw], lhs_ones,
                                     rhs3[:, kc, c0:c0 + cw],
                                     start=(kc == 0), stop=(kc == DO - 1))

        # Phase-C weight pool created ahead of the AB stack (LIFO pool order);
        # the DMAs are issued after phase A's loop so they don't compete with
        # A's critical loads.
        wc_stack = top.enter_context(ExitStack())
        wpoolC = wc_stack.enter_context(tc.tile_pool(name="wC", bufs=1))
        w1_sb = wpoolC.tile([P, DO, D], F16, tag="w1")
        w2_sb = wpoolC.tile([P, DO, D], F16, tag="w2")

        # ---------------- Phases A+B: QKV + KV (fp8 DoubleRow) ----------------
        with ExitStack() as ph:
            wpool = ph.enter_context(tc.tile_pool(name="wA", bufs=1))
            wq_sb = wpool.tile([P, DO, D], F8, tag="wq")
            nc.sync.dma_start(wq_sb, w8r["wq8"])
            kvres = ph.enter_context(tc.tile_pool(name="kvres", bufs=1))
            kp8 = kvres.tile([P, TOTCH, D], F8, tag="kp8")
            v8 = kvres.tile([P, TOTCH, D], F8, tag="v8")
            io = ph.enter_context(tc.tile_pool(name="ioA", bufs=2))
            x8_0 = io.tile([P, DO, NTA], F8, tag="x8A", name="x8_0")
            nc.sync.dma_start(x8_0, x8r[:, :, 0:NTA])
            wk_sb = wpool.tile([P, DO, D], F8, tag="wk")
            nc.sync.dma_start(wk_sb, w8r["wk8"])
            wv_sb = wpool.tile([P, DO, D], F8, tag="wv")
            nc.sync.dma_start(wv_sb, w8r["wv8"])
            if use_bq or use_bk or use_bv:
                nc.sync.dma_start(rows, rows_d[:])
            nc.sync.dma_start(pp, pp_d[:])
            nc.sync.dma_start(diags, diags_d[:])

            ev = ph.enter_context(tc.tile_pool(name="evA", bufs=4))
            pa = ph.enter_context(ExitStack())
            psQ = pa.enter_context(tc.tile_pool(name="psQA", bufs=1, space="PSUM"))
            psK = pa.enter_context(tc.tile_pool(name="psKA", bufs=2, space="PSUM"))
            psV = pa.enter_context(tc.tile_pool(name="psVA", bufs=2, space="PSUM"))

            def do_q(st, x8_t, n0):
                for dc in range(DO):
                    ps_q = psQ.tile([P, NTA], F32, tag="psq", name=f"psq{st}_{dc}")
                    for kcp in range(DO // 2):
                        ks2 = slice(2 * kcp, 2 * kcp + 2)
                        for h in range(2):
                            hs = slice(h * 512, (h + 1) * 512)
                            nc.tensor.matmul(ps_q[:, hs],
                                             wq_sb[:, ks2, dc * P:(dc + 1) * P],
                                             x8_t[:, ks2, hs],
                                             start=(kcp == 0),
                                             stop=(kcp == DO // 2 - 1 and not use_bq),
                                             perf_mode=DR)
                    if use_bq:
                        for h in range(2):
                            hs = slice(h * 512, (h + 1) * 512)
                            nc.tensor.matmul(ps_q[:, hs],
                                             rows[0:1, dc * P:(dc + 1) * P],
                                             ones_row[0:1, hs], start=False,
                                             stop=True)
                    for h in range(2):
                        hs = slice(h * 512, (h + 1) * 512)
                        # e_all = WS*exp(q); e1 = min(e_all, WS) = WS*exp(min(q,0))
                        e_all = ev.tile([P, 512], BF16, tag="eQ")
                        nc.scalar.activation(e_all, ps_q[:, hs], AF.Exp,
                                             scale=1.0 / WS, bias=lnws[:, 0:1])
                        e1 = ev.tile([P, 512], BF16, tag="e1Q")
                        nc.vector.tensor_scalar_min(e1, e_all, float(WS))
                        # qp' = max(WS*q, 0) + WS*exp(min(q,0)) = WS*Qp
                        nc.vector.scalar_tensor_tensor(
                            qp[:, dc, n0 + h * 512:n0 + (h + 1) * 512],
                            ps_q[:, hs], 0.0, e1, OP.max, OP.add)

            def do_kv(st, x8_t, n0):
                for ch in range(NCH_A):
                    cs = slice(ch * P, (ch + 1) * P)
                    chg = st * NCH_A + ch
                    for h in range(2):
                        hs = slice(h * 512, (h + 1) * 512)
                        ps_k = psK.tile([P, 512], F32, tag="psk")
                        ps_v = psV.tile([P, 512], F32, tag="psv")
                        for kcp in range(DO // 2):
                            ks2 = slice(2 * kcp, 2 * kcp + 2)
                            nc.tensor.matmul(ps_k, x8_t[:, ks2, cs],
                                             wk_sb[:, ks2, hs],
                                             start=(kcp == 0),
                                             stop=(kcp == DO // 2 - 1 and not use_bk),
                                             perf_mode=DR)
                            nc.tensor.matmul(ps_v, x8_t[:, ks2, cs],
                                             wv_sb[:, ks2, hs],
                                             start=(kcp == 0),
                                             stop=(kcp == DO // 2 - 1 and not use_bv),
                                             perf_mode=DR)
                        if use_bk:
                            nc.tensor.matmul(ps_k, ones_1p_bf[0:1, :],
                                             rows[0:1, D + h * 512:D + (h + 1) * 512],
                                             start=False, stop=True)
                        if use_bv:
                            nc.tensor.matmul(ps_v, ones_1p_bf[0:1, :],
                                             rows[0:1, 2 * D + h * 512:2 * D + (h + 1) * 512],
                                             start=False, stop=True)
                        # kp8 = KS*Kp = max(KS*k, 0) + min(KS*e^k, KS)
                        ek = ev.tile([P, 512], BF16, tag="eK")
                        nc.scalar.activation(ek, ps_k, AF.Exp,
                                             scale=1.0 / KS, bias=lnks[:, 0:1])
                        e1k = ev.tile([P, 512], BF16, tag="e1K")
                        nc.vector.tensor_scalar_min(e1k, ek, float(KS))
                        nc.vector.scalar_tensor_tensor(kp8[:, chg, hs], ps_k, 0.0,
                                                       e1k, OP.max, OP.add)
                        nc.scalar.activation(v8[:, chg, hs], ps_v, AF.Copy,
                                             scale=1.0 / WS)


            for st in range(NST):
                n0 = st * NTA
                if st == 0:
                    x8_t = x8_0
                else:
                    x8_t = io.tile([P, DO, NTA], F8, tag="x8A", name=f"x8_{st}")
                    nc.sync.dma_start(x8_t, x8r[:, :, n0:n0 + NTA])
                if st < NST - 1:
                    do_q(st, x8_t, n0)
                    do_kv(st, x8_t, n0)
                else:
                    # last supertile: K/V first so phase B's inputs finish early
                    do_kv(st, x8_t, n0)
                    do_q(st, x8_t, n0)

            # prefetch phase-C weights while A/B compute
            nc.sync.dma_start(w1_sb, wr["w1T"])
            nc.sync.dma_start(w2_sb, wr["w2T"])

            pa.close()

            # ---------------- Phase B: KV accumulation (fp8 DR) ----------------
            psB = ph.enter_context(tc.tile_pool(name="psB", bufs=1, space="PSUM"))
            for pass_ in range(4):
                kv_ps = [psB.tile([P, NTA], F32, tag=f"kvps{i}", name=f"kvps{pass_}_{i}")
                         for i in range(2)]
                for chp in range(TOTCH // 2):
                    c2 = slice(2 * chp, 2 * chp + 2)
                    for i in range(2):
                        dc = pass_ * 2 + i
                        for h in range(2):
                            hs = slice(h * 512, (h + 1) * 512)
                            nc.tensor.matmul(kv_ps[i][:, hs],
                                             kp8[:, c2, dc * P:(dc + 1) * P],
                                             v8[:, c2, hs],
                                             start=(chp == 0),
                                             stop=(chp == TOTCH // 2 - 1),
                                             perf_mode=DR)
                for i in range(2):
                    nc.scalar.activation(kv_sb[:, pass_ * 2 + i, :], kv_ps[i],
                                         AF.Copy, scale=1.0 / KS)

            # K_sum over all tokens (fp8 DR, from long-written kp8), then
            # transpose to per-partition column layout [P, DO]
            psks = ph.enter_context(tc.tile_pool(name="psks", bufs=1, space="PSUM"))
            ps_ks = [psks.tile([1, 512], F32, tag=f"ksr{h}", name=f"ksr{h}")
                     for h in range(2)]
            for chp in range(TOTCH // 2):
                c2 = slice(2 * chp, 2 * chp + 2)
                for h in range(2):
                    nc.tensor.matmul(ps_ks[h], ones8[:, :, 0:1],
                                     kp8[:, c2, h * 512:(h + 1) * 512],
                                     start=(chp == 0), stop=(chp == TOTCH // 2 - 1),
                                     perf_mode=DR)
            for h in range(2):
                hs = slice(h * 512, (h + 1) * 512)
                nc.scalar.activation(ksrow_sb[0:1, hs], ps_ks[h], AF.Copy,
                                     scale=1.0 / KS)
            ps_ksc = psks.tile([P, DO], F32, tag="kscol")
            for dc in range(DO):
                nc.tensor.matmul(ps_ksc[:, dc:dc + 1],
                                 ksrow_sb[0:1, dc * P:(dc + 1) * P],
                                 ones_one[0:1, 0:1], start=True, stop=True)
            nc.scalar.activation(ksum_sb[:, :, 0], ps_ksc, AF.Copy)

        # ---------------- Phase C: conv'' + local MLP + token mixer ----------------
        with ExitStack() as ph:
            io = ph.enter_context(tc.tile_pool(name="ioC", bufs=2))
            pipe = ph.enter_context(tc.tile_pool(name="pipeC", bufs=2))
            mid = ph.enter_context(tc.tile_pool(name="midC", bufs=1))
            sm = ph.enter_context(tc.tile_pool(name="smC", bufs=1))
            ps = ph.enter_context(tc.tile_pool(name="psC", bufs=2, space="PSUM"))
            pst = ph.enter_context(tc.tile_pool(name="pstC", bufs=1, space="PSUM"))

            def c_front(it):
                n0 = it * NT
                x_t = io.tile([P, DO, W], F16, tag="xC", name=f"x_{it}")
                lo, hi = n0 - 2, n0 + NT + 2
                if lo < 0:
                    nc.vector.memset(x_t[:, :, 0:2], 0.0)
                    nc.sync.dma_start(x_t[:, :, 2:W], xT[:, :, 0:hi])
                elif hi > N:
                    nc.vector.memset(x_t[:, :, W - 2:W], 0.0)
                    nc.sync.dma_start(x_t[:, :, 0:W - 2], xT[:, :, lo:N])
                else:
                    nc.sync.dma_start(x_t, xT[:, :, lo:hi])

                dcv = io.tile([P, DO, NT], F16, tag="dcvC", name=f"dcv_{it}")
                # diffusion dwconv'': center tap on ACT, side taps on DVE
                for o in range(DO):
                    nc.scalar.activation(dcv[:, o, :], x_t[:, o, 2:NT + 2],
                                         AF.Identity, bias=pp[:, o, CB:CB + 1],
                                         scale=pp[:, o, C1:C1 + 1])
                for o in range(DO):
                    nc.vector.scalar_tensor_tensor(dcv[:, o, :], x_t[:, o, 1:NT + 1],
                                                   pp[:, o, C0:C0 + 1], dcv[:, o, :],
                                                   OP.mult, OP.add)
                for o in range(DO):
                    nc.vector.scalar_tensor_tensor(dcv[:, o, :], x_t[:, o, 3:NT + 3],
                                                   pp[:, o, C2:C2 + 1], dcv[:, o, :],
                                                   OP.mult, OP.add)

                # local MLP first half
                h1_t = pipe.tile([P, DO, NT], F16, tag="h1", name=f"h1_{it}")
                for dc in range(DO):
                    ps_h = ps.tile([P, NT], F32, tag="psh1", name=f"psh1_{it}_{dc}")
                    for kc in range(DO):
                        nc.tensor.matmul(ps_h, w1_sb[:, kc, dc * P:(dc + 1) * P],
                                         x_t[:, kc, 2:NT + 2],
                                         start=(kc == 0), stop=(kc == DO - 1))
                    nc.scalar.activation(h1_t[:, dc, :], ps_h, AF.Gelu,
                                         bias=pp[:, dc, LUB1:LUB1 + 1])

                # token mixer LN stats
                sq_t = mid.tile([P, DO, W], F16, tag="tokA", name=f"sq_{it}")
                nc.scalar.activation(sq_t, x_t, AF.Square)
                ps_m = pst.tile([P, W], F32, tag="psm", name=f"psm_{it}")
                stats_mm(ps_m, onesD, x_t, W)
                ps_s = pst.tile([P, W], F32, tag="pss", name=f"pss_{it}")
                stats_mm(ps_s, onesD, sq_t, W)
                m_sb = sm.tile([P, W], F16, tag="msb", name=f"msb_{it}")
                nc.scalar.activation(m_sb, ps_m, AF.Copy)
                var = sm.tile([P, W], F32, tag="var", name=f"var_{it}")
                nc.scalar.activation(var, ps_m, AF.Square)
                nc.vector.tensor_sub(var, ps_s, var)
                nc.scalar.activation(var, var, AF.Sqrt, bias=eps_ln[:, 0:1])
                nc.vector.reciprocal_approx_fast(out=var, in_=var)
                rstd = sm.tile([P, W], F16, tag="rstd", name=f"rstd_{it}")
                nc.vector.tensor_copy(rstd, var)
                u_t = mid.tile([P, DO, W], F16, tag="tokA", name=f"u_{it}")
                for o in range(DO):
                    nc.vector.tensor_sub(u_t[:, o, :], x_t[:, o, :], m_sb)
                xm_t = mid.tile([P, DO, W], F16, tag="tokC", name=f"xm_{it}")
                for o in range(DO):
                    nc.vector.scalar_tensor_tensor(xm_t[:, o, :], u_t[:, o, :],
                                                   pp[:, o, TMG:TMG + 1], rstd,
                                                   OP.mult, OP.mult)
                if use_tmb:
                    for o in range(DO):
                        nc.vector.tensor_scalar_add(xm_t[:, o, :], xm_t[:, o, :],
                                                    pp[:, o, TMB:TMB + 1])
                # conv1: t_s[k] = conv1(xm)[k+1], k in [0, W-2)
                t_t = mid.tile([P, DO, W - 2], F16, tag="tokD", name=f"t_{it}")
                for o in range(DO):
                    nc.scalar.activation(t_t[:, o, :], xm_t[:, o, 1:W - 1],
                                         AF.Identity, bias=pp[:, o, TCB1:TCB1 + 1],
                                         scale=pp[:, o, T1:T1 + 1])
                for o in range(DO):
                    nc.vector.scalar_tensor_tensor(t_t[:, o, :], xm_t[:, o, 0:W - 2],
                                                   pp[:, o, T0:T0 + 1],
                                                   t_t[:, o, :], OP.mult, OP.add)
                for o in range(DO):
                    nc.vector.scalar_tensor_tensor(t_t[:, o, :], xm_t[:, o, 2:W],
                                                   pp[:, o, T2:T2 + 1],
                                                   t_t[:, o, :], OP.mult, OP.add)
                t2_t = pipe.tile([P, DO, W - 2], F16, tag="t2", name=f"t2_{it}")
                nc.scalar.activation(t2_t, t_t, AF.Gelu)
                if it == 0:
                    nc.vector.memset(t2_t[:, :, 0:1], 0.0)
                if it == NTILES - 1:
                    nc.vector.memset(t2_t[:, :, W - 3:W - 2], 0.0)
                return x_t, dcv, h1_t, t2_t

            def c_back(it, tiles):
                n0 = it * NT
                x_t, dcv, h1_t, t2_t = tiles
                acc = io.tile([P, DO, NT], F16, tag="accC", name=f"acc_{it}")
                for dc in range(DO):
                    ps_h = ps.tile([P, NT], F32, tag="psh2", name=f"psh2_{it}_{dc}")
                    for kc in range(DO):
                        nc.tensor.matmul(ps_h, w2_sb[:, kc, dc * P:(dc + 1) * P],
                                         h1_t[:, kc, :],
                                         start=(kc == 0), stop=False)
                    for tap in range(3):
                        nc.tensor.matmul(ps_h, diags[:, tap, dc, :],
                                         t2_t[:, dc, tap:NT + tap],
                                         start=False, stop=(tap == 2))
                    nc.vector.tensor_add(acc[:, dc, :], ps_h, dcv[:, dc, :])
                nc.sync.dma_start(acc_r[:, :, n0:n0 + NT], acc)

            pend = {0: c_front(0)}
            for it in range(NTILES):
                if it + 1 < NTILES:
                    pend[it + 1] = c_front(it + 1)
                c_back(it, pend.pop(it))

        wc_stack.close()

        # ---------------- Phase D: attention + LN1 + FFN + LN2 ----------------
        # Pipelined; numerator of tile t+1 is split into two half-groups that
        # are issued under tile t's two DVE-bound LN chains.
        with ExitStack() as ph:
            wpoolD = ph.enter_context(tc.tile_pool(name="wD", bufs=1))
            f1_sb = wpoolD.tile([P, DO, D], F8, tag="f1")
            nc.sync.dma_start(f1_sb, w8r["f18"])
            f2_sb = wpoolD.tile([P, DO, D], F8, tag="f2")
            nc.sync.dma_start(f2_sb, w8r["f28"])
            io = ph.enter_context(tc.tile_pool(name="ioD", bufs=3))
            mid = ph.enter_context(tc.tile_pool(name="midD", bufs=1))
            bk = ph.enter_context(tc.tile_pool(name="bkD", bufs=2))
            sm = ph.enter_context(tc.tile_pool(name="smD", bufs=2))
            ps = ph.enter_context(tc.tile_pool(name="psD", bufs=2, space="PSUM"))
            psf_pool = ph.enter_context(tc.tile_pool(name="psfD", bufs=2, space="PSUM"))
            pst = ph.enter_context(tc.tile_pool(name="pstD", bufs=1, space="PSUM"))

            def d_load(it):
                n0 = it * NT
                acc_t = io.tile([P, DO, NT], F16, tag="accD", name=f"accD_{it}")
                nc.sync.dma_start(acc_t, acc_r[:, :, n0:n0 + NT])
                return acc_t

            def d_front_a(it, acc_t):
                """norm row, 1/norm fold, numerator halves 0-3."""
                n0 = it * NT
                ps_n = ps.tile([P, NT], F32, tag="psnum", name=f"psn_{it}")
                for kc in range(DO):
                    nc.tensor.matmul(ps_n[0:1, :], ksum_sb[:, kc, :],
                                     qp[:, kc, n0:n0 + NT],
                                     start=(kc == 0), stop=(kc == DO - 1))
                nr = sm.tile([1, NT], F32, tag="nrD", name=f"nr_{it}")
                nc.vector.tensor_scalar_add(nr, ps_n[0:1, :], 1e-6)
                rr = sm.tile([1, NT], F32, tag="rrD", name=f"rr_{it}")
                nc.vector.reciprocal_approx_fast(out=rr, in_=nr)
                ps_rep = ps.tile([P, NT], F32, tag="psnum", name=f"psrep_{it}")
                nc.tensor.matmul(ps_rep, ones_1p_f32[0:1, :], rr, start=True,
                                 stop=True)
                rep_sb = mid.tile([P, NT], BF16, tag="repsb", name=f"rep_{it}")
                nc.scalar.activation(rep_sb, ps_rep, AF.Copy)
                for kc in range(DO):
                    nc.vector.tensor_mul(qp[:, kc, n0:n0 + NT],
                                         qp[:, kc, n0:n0 + NT], rep_sb)
                for ec in range(DO // 2):
                    ps_u = ps.tile([P, NT], F32, tag="psnum", name=f"psnum_{it}_{ec}")
                    for kc in range(DO):
                        nc.tensor.matmul(ps_u, kv_sb[:, kc, ec * P:(ec + 1) * P],
                                         qp[:, kc, n0:n0 + NT],
                                         start=(kc == 0), stop=(kc == DO - 1))
                    nc.vector.tensor_add(acc_t[:, ec, :], acc_t[:, ec, :], ps_u)
                return acc_t

            def d_front_b(it, acc_t):
                n0 = it * NT
                for ec in range(DO // 2, DO):
                    ps_u = ps.tile([P, NT], F32, tag="psnum", name=f"psnum_{it}_{ec}")
                    for kc in range(DO):
                        nc.tensor.matmul(ps_u, kv_sb[:, kc, ec * P:(ec + 1) * P],
                                         qp[:, kc, n0:n0 + NT],
                                         start=(kc == 0), stop=(kc == DO - 1))
                    nc.vector.tensor_add(acc_t[:, ec, :], acc_t[:, ec, :], ps_u)
                return acc_t

            def d_mid(it, acc_t):
                """LN1 stats + apply -> y1 (fp16)."""
                sq_t = mid.tile([P, DO, NT], F16, tag="sqD", name=f"sqD_{it}")
                nc.scalar.activation(sq_t, acc_t, AF.Square)
                ps_m1 = pst.tile([P, NT], F32, tag="psm1", name=f"psm1_{it}")
                stats_mm(ps_m1, onesD, acc_t, NT)
                ps_s1 = pst.tile([P, NT], F32, tag="pss1", name=f"pss1_{it}")
                stats_mm(ps_s1, onesD, sq_t, NT)
                m1_sb = sm.tile([P, NT], F16, tag="m1sb", name=f"m1_{it}")
                nc.scalar.activation(m1_sb, ps_m1, AF.Copy)
                var1 = sm.tile([P, NT], F32, tag="varD", name=f"var1_{it}")
                nc.scalar.activation(var1, ps_m1, AF.Square)
                nc.vector.tensor_sub(var1, ps_s1, var1)
                nc.scalar.activation(var1, var1, AF.Sqrt, bias=eps_ln[:, 0:1])
                nc.vector.reciprocal_approx_fast(out=var1, in_=var1)
                rstd1 = sm.tile([P, NT], F16, tag="rstdb", name=f"rstdb_{it}")
                nc.vector.tensor_copy(rstd1, var1)
                u1_t = mid.tile([P, DO, NT], F16, tag="u1", name=f"u1_{it}")
                for o in range(DO):
                    nc.vector.tensor_sub(u1_t[:, o, :], acc_t[:, o, :], m1_sb)
                y1_t = mid.tile([P, DO, NT], F16, tag="y1", name=f"y1_{it}")
                for o in range(DO):
                    nc.vector.scalar_tensor_tensor(y1_t[:, o, :], u1_t[:, o, :],
                                                   pp[:, o, N1G:N1G + 1], rstd1,
                                                   OP.mult, OP.mult)
                if use_n1b:
                    for o in range(DO):
                        nc.vector.tensor_scalar_add(y1_t[:, o, :], y1_t[:, o, :],
                                                    pp[:, o, N1B:N1B + 1])
                return y1_t

            def d_ffn(it, y1_t):
                # fp8 cast of y1 for the DR matmuls (residual stays fp16)
                y18_t = mid.tile([P, DO, NT], F8, tag="y18", name=f"y18_{it}")
                nc.scalar.activation(y18_t, y1_t, AF.Copy)
                f1h_t = mid.tile([P, DO, NT], F8, tag="f1h", name=f"f1h_{it}")
                for dc in range(DO):
                    ps_f = psf_pool.tile([P, NT], F32, tag="psf",
                                         name=f"psf1_{it}_{dc}")
                    for kcp in range(DO // 2):
                        ks2 = slice(2 * kcp, 2 * kcp + 2)
                        nc.tensor.matmul(ps_f, f1_sb[:, ks2, dc * P:(dc + 1) * P],
                                         y18_t[:, ks2, :],
                                         start=(kcp == 0), stop=(kcp == DO // 2 - 1),
                                         perf_mode=DR)
                    nc.scalar.activation(f1h_t[:, dc, :], ps_f, AF.Gelu,
                                         scale=1.0 / WS,
                                         bias=pp[:, dc, FFB1:FFB1 + 1])
                y2_t = bk.tile([P, DO, NT], F16, tag="y2", name=f"y2_{it}")
                for dc in range(DO):
                    ps_f = psf_pool.tile([P, NT], F32, tag="psf",
                                         name=f"psf2_{it}_{dc}")
                    for kcp in range(DO // 2):
                        ks2 = slice(2 * kcp, 2 * kcp + 2)
                        nc.tensor.matmul(ps_f, f2_sb[:, ks2, dc * P:(dc + 1) * P],
                                         f1h_t[:, ks2, :],
                                         start=(kcp == 0), stop=(kcp == DO // 2 - 1),
                                         perf_mode=DR)
                    nc.vector.scalar_tensor_tensor(y2_t[:, dc, :], ps_f,
                                                   1.0 / WS,
                                                   y1_t[:, dc, :], OP.mult, OP.add)
                    if use_fb2:
                        nc.scalar.activation(y2_t[:, dc, :], y2_t[:, dc, :],
                                             AF.Identity,
                                             bias=pp[:, dc, FFB2:FFB2 + 1])
                return y2_t

            def d_back(it, y2_t):
                n0 = it * NT
                sq2_t = bk.tile([P, DO, NT], F16, tag="sq2", name=f"sq2_{it}")
                nc.scalar.activation(sq2_t, y2_t, AF.Square)
                ps_m2 = pst.tile([P, NT], F32, tag="psm2", name=f"psm2_{it}")
                stats_mm(ps_m2, onesD, y2_t, NT)
                ps_s2 = pst.tile([P, NT], F32, tag="pss2", name=f"pss2_{it}")
                stats_mm(ps_s2, onesD, sq2_t, NT)
                m2_sb = sm.tile([P, NT], F32, tag="m2sb", name=f"m2_{it}")
                nc.scalar.activation(m2_sb, ps_m2, AF.Copy)
                var2 = sm.tile([P, NT], F32, tag="varD", name=f"var2_{it}")
                nc.scalar.activation(var2, ps_m2, AF.Square)
                nc.vector.tensor_sub(var2, ps_s2, var2)
                nc.scalar.activation(var2, var2, AF.Sqrt, bias=eps_ln[:, 0:1])
                nc.vector.reciprocal_approx_fast(out=var2, in_=var2)
                yo_t = bk.tile([P, DO, NT], F16, tag="yo", name=f"yo_{it}")
                for o in range(DO):
                    nc.vector.tensor_sub(yo_t[:, o, :], y2_t[:, o, :], m2_sb)
                for o in range(DO):
                    nc.vector.scalar_tensor_tensor(yo_t[:, o, :], yo_t[:, o, :],
                                                   pp[:, o, N2G:N2G + 1], var2,
                                                   OP.mult, OP.mult)
                if use_n2b:
                    for o in range(DO):
                        nc.vector.tensor_scalar_add(yo_t[:, o, :], yo_t[:, o, :],
                                                    pp[:, o, N2B:N2B + 1])
                nc.sync.dma_start(yT[:, :, n0:n0 + NT], yo_t)

            acc_tiles = {0: d_load(0)}
            if NTILES > 1:
                acc_tiles[1] = d_load(1)
            acc_cur = d_front_b(0, d_front_a(0, acc_tiles.pop(0)))
            acc_nxt = None
            y2_prev = None
            # d_back runs one slot late so its ACT/DVE chains overlap the
            # next tile's d_mid/d_ffn matmuls
            for it in range(NTILES):
                y1_cur = d_mid(it, acc_cur)
                if it + 2 < NTILES:
                    acc_tiles[it + 2] = d_load(it + 2)
                if it + 1 < NTILES:
                    acc_nxt = d_front_a(it + 1, acc_tiles.pop(it + 1))
                y2_cur = d_ffn(it, y1_cur)
                if it + 1 < NTILES:
                    acc_cur = d_front_b(it + 1, acc_nxt)
                if y2_prev is not None:
                    d_back(it - 1, y2_prev)
                y2_prev = y2_cur
            d_back(NTILES - 1, y2_prev)

    nc.compile()
    return nc


def make_in_maps(inputs, n_cores=8):
    """Host-side preprocessing: fold constants, transpose, cast, shard."""
    x = np.asarray(inputs["x"], np.float32)
    B, N, D_ = x.shape
    dt = float(np.asarray(inputs["delta_t"]))

    def g(k):
        return np.asarray(inputs[k], np.float32)

    diff_w, diff_b = g("diff_w"), g("diff_b")
    tm_w1, tm_cb1 = g("tm_w1"), g("tm_cb1")
    tm_w2, tm_cb2 = g("tm_w2"), g("tm_cb2")

    pp = np.zeros((P, DO, NPARAM), np.float32)

    def put(i, v):
        pp[:, :, i] = v.reshape(DO, P).T

    put(C0, dt * diff_w[:, 0, 0])
    put(C1, dt * diff_w[:, 0, 1] + (1.0 - dt))
    put(C2, dt * diff_w[:, 0, 2])
    put(CB, dt * diff_b + g("lu_b2") + tm_cb2)
    put(T0, tm_w1[:, 0, 0])
    put(T1, tm_w1[:, 0, 1])
    put(T2, tm_w1[:, 0, 2])
    put(TCB1, tm_cb1)
    put(U0, tm_w2[:, 0, 0])
    put(U1, tm_w2[:, 0, 1])
    put(U2, tm_w2[:, 0, 2])
    put(TMG, g("tm_g"))
    put(TMB, g("tm_beta"))
    put(N1G, g("n1_g"))
    put(N1B, g("n1_b"))
    put(N2G, g("n2_g"))
    put(N2B, g("n2_b"))
    put(LUB1, g("lu_b1"))
    put(FFB1, g("ff_b1"))
    put(FFB2, g("ff_b2"))

    diags = np.zeros((P, 3, DO, P), np.float32)
    idx = np.arange(P)
    for tap in range(3):
        for dc in range(DO):
            diags[idx, tap, dc, idx] = tm_w2[dc * P + idx, 0, tap]
    diags = diags.astype(F16_NP)

    rows = np.zeros((1, 3 * D), np.float32)
    rows[0, 0:D] = g("bq") * WS
    rows[0, D:2 * D] = g("bk") * KS
    rows[0, 2 * D:3 * D] = g("bv") * WS
    rows = rows.astype(BF16_NP)

    wt = {}
    for name, key in (("w1T", "lu_w1"), ("w2T", "lu_w2")):
        wt[name] = np.ascontiguousarray(g(key).T).astype(F16_NP)
    w8 = {}
    for name, key, s in (("wq8", "wq", WS), ("wk8", "wk", KS), ("wv8", "wv", WS),
                         ("f18", "ff_w1", WS), ("f28", "ff_w2", WS)):
        w8[name] = np.ascontiguousarray(g(key).T * s).astype(F8_NP)

    xT = np.ascontiguousarray(x.transpose(0, 2, 1)).astype(F16_NP)
    x8 = xT.astype(F8_NP)

    flags = dict(
        use_bq=bool(np.any(g("bq"))),
        use_bk=bool(np.any(g("bk"))),
        use_bv=bool(np.any(g("bv"))),
        use_tmb=bool(np.any(g("tm_beta"))),
        use_n1b=bool(np.any(g("n1_b"))),
        use_n2b=bool(np.any(g("n2_b"))),
        use_fb2=bool(np.any(g("ff_b2"))),
    )

    shared = {**wt, **w8, "pp": pp, "rows": rows, "diags": diags}
    in_maps = [{**shared, "x_T": xT[b], "x_8": x8[b]} for b in range(B)]
    return in_maps, flags, (B, N)


_NC_CACHE = {}


def kernel(**inputs):
    in_maps, flags, (B, N) = make_in_maps(inputs)
    key = (N, tuple(sorted(flags.items())))
    if key not in _NC_CACHE:
        _NC_CACHE[key] = build_nc(N=N, NT=512, **flags)
    nc = _NC_CACHE[key]
    res = run_bass_kernel_spmd(nc, in_maps, list(range(B)))
    y = np.stack([res.results[b]["y_T"] for b in range(B)])
    return np.ascontiguousarray(y.transpose(0, 2, 1)).astype(np.float32)


# revision 47
# speedup vs baseline: 1.1148x; 1.0217x over previous
"""Trainium2 Bass kernel for nn_DiffuRNNLayer (B=8, N=2048, D=1024).

Sharding: data-parallel over batch - one batch element per NeuronCore (8 cores).
Per-core phases:
  A: Q/K/V projections in fp8 DoubleRow (+elu+1); Qp (bf16, scaled by WS),
     Kp (fp8, x16) / V (fp8) SBUF-resident; K_sum accumulated inline.
  B: KV = Kp^T V via fp8 DoubleRow from SBUF-resident Kp/V.
  C: acc = dwconv''(x) + MLP(x) + tokenmixer(LN(x)) in fp16; spill acc.
  D: attn numerator with 1/norm folded into Qp, acc += attn; LN1; FFN
     residual; LN2; write y^T (fp16).
"""

import numpy as np
import ml_dtypes
from contextlib import ExitStack

import concourse.bass as bass
import concourse.bacc as bacc
import concourse.tile as tile
import concourse.mybir as mybir
from concourse.bass_utils import run_bass_kernel_spmd

F32 = mybir.dt.float32
BF16 = mybir.dt.bfloat16
F16 = mybir.dt.float16
F8 = mybir.dt.float8e4
AF = mybir.ActivationFunctionType
OP = mybir.AluOpType
DR = mybir.MatmulPerfMode.DoubleRow
BF16_NP = ml_dtypes.bfloat16
F16_NP = np.float16
F8_NP = ml_dtypes.float8_e4m3

P = 128
D = 1024
DO = D // P  # 8 chunks of the channel dim
WS = 256.0   # fp8 weight scale for wq/wv
KS = 16.0    # fp8 weight scale for wk (kp8 = KS*Kp must stay under 240)
LNWS = float(np.log(WS))
LNKS = float(np.log(KS))

# pp param-plane indices (per-partition params, laid out [128, DO, NP])
(C0, C1, C2, CB, T0, T1, T2, TCB1, U0, U1, U2,
 TMG, TMB, N1G, N1B, N2G, N2B, LUB1, FFB1, FFB2) = range(20)
NPARAM = 20


def build_nc(N=2048, NT=512, use_bq=False, use_bk=False, use_bv=False,
             use_tmb=False, use_n1b=False, use_n2b=False, use_fb2=False,
             debug=False):
    NTILES = N // NT
    NTA = 1024             # phase-A supertile width
    NST = N // NTA
    NCH_A = NTA // P       # 128-token chunks per supertile
    TOTCH = N // P
    W = NT + 4             # phase-C tile width with +-2 halo
    assert N % NT == 0 and NT % P == 0 and N % NTA == 0

    nc = bacc.Bacc(None, target_bir_lowering=False, debug=debug)

    xT_d = nc.dram_tensor("x_T", [D, N], F16, kind="ExternalInput")
    x8_d = nc.dram_tensor("x_8", [D, N], F8, kind="ExternalInput")
    w8_d = {}
    for name in ("wq8", "wk8", "wv8"):
        w8_d[name] = nc.dram_tensor(name, [D, D], F8, kind="ExternalInput")
    w_d = {}
    for name in ("w1T", "w2T"):
        w_d[name] = nc.dram_tensor(name, [D, D], F16, kind="ExternalInput")
    for name in ("f18", "f28"):
        w8_d[name] = nc.dram_tensor(name, [D, D], F8, kind="ExternalInput")
    pp_d = nc.dram_tensor("pp", [P, DO, NPARAM], F32, kind="ExternalInput")
    diags_d = nc.dram_tensor("diags", [P, 3, DO, P], F16, kind="ExternalInput")
    rows_d = nc.dram_tensor("rows", [1, 3 * D], BF16, kind="ExternalInput")
    yT_d = nc.dram_tensor("y_T", [D, N], F16, kind="ExternalOutput")

    acc_sp = nc.dram_tensor("acc_sp", [D, N], F16)

    xT = xT_d.rearrange("(o p) n -> p o n", p=P)
    x8r = x8_d.rearrange("(o p) n -> p o n", p=P)
    w8r = {k: v.rearrange("(o p) n -> p o n", p=P) for k, v in w8_d.items()}
    wr = {k: v.rearrange("(o p) n -> p o n", p=P) for k, v in w_d.items()}
    acc_r = acc_sp.rearrange("(o p) n -> p o n", p=P)
    yT = yT_d.rearrange("(o p) n -> p o n", p=P)

    with tile.TileContext(nc) as tc, ExitStack() as top:
        persist = top.enter_context(tc.tile_pool(name="persist", bufs=1))
        ones_1p_f32 = persist.tile([1, P], F32)
        nc.vector.memset(ones_1p_f32, 1.0)
        ones_one = persist.tile([1, 1], BF16)
        nc.vector.memset(ones_one, 1.0)
        ones8 = persist.tile([P, 2, 16], F8)
        nc.vector.memset(ones8, 1.0)
        ksrow_sb = persist.tile([1, D], BF16)
        onesD = persist.tile([P, P], F16)
        nc.vector.memset(onesD, 1.0 / D)
        eps_ln = persist.tile([P, 1], F32)
        nc.vector.memset(eps_ln, 1e-5)
        lnws = persist.tile([P, 1], F32)
        nc.vector.memset(lnws, LNWS)
        lnks = persist.tile([P, 1], F32)
        nc.vector.memset(lnks, LNKS)
        kv_sb = persist.tile([P, DO, D], BF16)
        ksum_sb = persist.tile([P, DO, 1], BF16)
        qp = persist.tile([P, DO, N], BF16)  # WS-scaled Qp, resident
        pp = persist.tile([P, DO, NPARAM], F32)
        diags = persist.tile([P, 3, DO, P], F16)
        rows = ones_row = ones_1p_bf = None
        if use_bq or use_bk or use_bv:
            rows = persist.tile([1, 3 * D], BF16)
            ones_row = persist.tile([1, NTA], BF16)
            nc.vector.memset(ones_row, 1.0)
            ones_1p_bf = persist.tile([1, P], BF16)
            nc.vector.memset(ones_1p_bf, 1.0)

        def stats_mm(psum, lhs_ones, rhs3, width):
            """Accumulate over DO k-chunks: psum[:, j] = mean over channel dim,
            replicated across partitions.  rhs3: [P, DO, width]."""
            for c0 in range(0, width, 512):
                cw = min(512, width - c0)
                for kc in range(DO):
                    nc.tensor.matmul(psum[:, c0:c0 + cw], lhs_ones,
                                     rhs3[:, kc, c0:c0 + cw],
                                     start=(kc == 0), stop=(kc == DO - 1))

        # Phase-C weight pool created ahead of the AB stack (LIFO pool order);
        # the DMAs are issued after phase A's loop so they don't compete with
        # A's critical loads.
        wc_stack = top.enter_context(ExitStack())
        wpoolC = wc_stack.enter_context(tc.tile_pool(name="wC", bufs=1))
        w1_sb = wpoolC.tile([P, DO, D], F16, tag="w1")
        w2_sb = wpoolC.tile([P, DO, D], F16, tag="w2")

        # ---------------- Phases A+B: QKV + KV (fp8 DoubleRow) ----------------
        with ExitStack() as ph:
            wpool = ph.enter_context(tc.tile_pool(name="wA", bufs=1))
            wq_sb = wpool.tile([P, DO, D], F8, tag="wq")
            nc.sync.dma_start(wq_sb, w8r["wq8"])
            kvres = ph.enter_context(tc.tile_pool(name="kvres", bufs=1))
            kp8 = kvres.tile([P, TOTCH, D], F8, tag="kp8")
            v8 = kvres.tile([P, TOTCH, D], F8, tag="v8")
            io = ph.enter_context(tc.tile_pool(name="ioA", bufs=2))
            x8_0 = io.tile([P, DO, NTA], F8, tag="x8A", name="x8_0")
            nc.sync.dma_start(x8_0, x8r[:, :, 0:NTA])
            wk_sb = wpool.tile([P, DO, D], F8, tag="wk")
            nc.sync.dma_start(wk_sb, w8r["wk8"])
            wv_sb = wpool.tile([P, DO, D], F8, tag="wv")
            nc.sync.dma_start(wv_sb, w8r["wv8"])
            if use_bq or use_bk or use_bv:
                nc.sync.dma_start(rows, rows_d[:])
            nc.sync.dma_start(pp, pp_d[:])
            nc.sync.dma_start(diags, diags_d[:])

            ev = ph.enter_context(tc.tile_pool(name="evA", bufs=4))
            pa = ph.enter_context(ExitStack())
            psQ = pa.enter_context(tc.tile_pool(name="psQA", bufs=1, space="PSUM"))
            psK = pa.enter_context(tc.tile_pool(name="psKA", bufs=3, space="PSUM"))
            psV = pa.enter_context(tc.tile_pool(name="psVA", bufs=3, space="PSUM"))

            def do_q(st, x8_t, n0):
                for dc in range(DO):
                    ps_q = psQ.tile([P, NTA], F32, tag="psq", name=f"psq{st}_{dc}")
                    for kcp in range(DO // 2):
                        ks2 = slice(2 * kcp, 2 * kcp + 2)
                        for h in range(2):
                            hs = slice(h * 512, (h + 1) * 512)
                            nc.tensor.matmul(ps_q[:, hs],
                                             wq_sb[:, ks2, dc * P:(dc + 1) * P],
                                             x8_t[:, ks2, hs],
                                             start=(kcp == 0),
                                             stop=(kcp == DO // 2 - 1 and not use_bq),
                                             perf_mode=DR)
                    if use_bq:
                        for h in range(2):
                            hs = slice(h * 512, (h + 1) * 512)
                            nc.tensor.matmul(ps_q[:, hs],
                                             rows[0:1, dc * P:(dc + 1) * P],
                                             ones_row[0:1, hs], start=False,
                                             stop=True)
                    # e_all = WS*exp(q); e1 = min(e_all, WS) = WS*exp(min(q,0))
                    e_all = ev.tile([P, NTA], BF16, tag="eQ")
                    nc.scalar.activation(e_all, ps_q, AF.Exp,
                                         scale=1.0 / WS, bias=lnws[:, 0:1])
                    e1 = ev.tile([P, NTA], BF16, tag="e1Q")
                    nc.vector.tensor_scalar_min(e1, e_all, float(WS))
                    # qp' = max(WS*q, 0) + WS*exp(min(q,0)) = WS*Qp
                    nc.vector.scalar_tensor_tensor(
                        qp[:, dc, n0:n0 + NTA],
                        ps_q, 0.0, e1, OP.max, OP.add)

            def do_kv(st, x8_t, n0):
                for ch in range(NCH_A):
                    cs = slice(ch * P, (ch + 1) * P)
                    chg = st * NCH_A + ch
                    for h in range(2):
                        hs = slice(h * 512, (h + 1) * 512)
                        ps_k = psK.tile([P, 512], F32, tag="psk")
                        ps_v = psV.tile([P, 512], F32, tag="psv")
                        for kcp in range(DO // 2):
                            ks2 = slice(2 * kcp, 2 * kcp + 2)
                            nc.tensor.matmul(ps_k, x8_t[:, ks2, cs],
                                             wk_sb[:, ks2, hs],
                                             start=(kcp == 0),
                                             stop=(kcp == DO // 2 - 1 and not use_bk),
                                             perf_mode=DR)
                            nc.tensor.matmul(ps_v, x8_t[:, ks2, cs],
                                             wv_sb[:, ks2, hs],
                                             start=(kcp == 0),
                                             stop=(kcp == DO // 2 - 1 and not use_bv),
                                             perf_mode=DR)
                        if use_bk:
                            nc.tensor.matmul(ps_k, ones_1p_bf[0:1, :],
                                             rows[0:1, D + h * 512:D + (h + 1) * 512],
                                             start=False, stop=True)
                        if use_bv:
                            nc.tensor.matmul(ps_v, ones_1p_bf[0:1, :],
                                             rows[0:1, 2 * D + h * 512:2 * D + (h + 1) * 512],
                                             start=False, stop=True)
                        # kp8 = KS*Kp = max(KS*k, 0) + min(KS*e^k, KS)
                        ek = ev.tile([P, 512], BF16, tag="eK")
                        nc.scalar.activation(ek, ps_k, AF.Exp,
                                             scale=1.0 / KS, bias=lnks[:, 0:1])
                        e1k = ev.tile([P, 512], BF16, tag="e1K")
                        nc.vector.tensor_scalar_min(e1k, ek, float(KS))
                        nc.vector.scalar_tensor_tensor(kp8[:, chg, hs], ps_k, 0.0,
                                                       e1k, OP.max, OP.add)
                        nc.scalar.activation(v8[:, chg, hs], ps_v, AF.Copy,
                                             scale=1.0 / WS)


            for st in range(NST):
                n0 = st * NTA
                if st == 0:
                    x8_t = x8_0
                else:
                    x8_t = io.tile([P, DO, NTA], F8, tag="x8A", name=f"x8_{st}")
                    nc.sync.dma_start(x8_t, x8r[:, :, n0:n0 + NTA])
                if st < NST - 1:
                    do_q(st, x8_t, n0)
                    do_kv(st, x8_t, n0)
                else:
                    # last supertile: K/V first so phase B's inputs finish early
                    do_kv(st, x8_t, n0)
                    do_q(st, x8_t, n0)

            # prefetch phase-C weights while A/B compute
            nc.sync.dma_start(w1_sb, wr["w1T"])
            nc.sync.dma_start(w2_sb, wr["w2T"])

            pa.close()

            # ---------------- Phase B: KV accumulation (fp8 DR) ----------------
            psB = ph.enter_context(tc.tile_pool(name="psB", bufs=1, space="PSUM"))
            for pass_ in range(4):
                kv_ps = [psB.tile([P, NTA], F32, tag=f"kvps{i}", name=f"kvps{pass_}_{i}")
                         for i in range(2)]
                for chp in range(TOTCH // 2):
                    c2 = slice(2 * chp, 2 * chp + 2)
                    for i in range(2):
                        dc = pass_ * 2 + i
                        for h in range(2):
                            hs = slice(h * 512, (h + 1) * 512)
                            nc.tensor.matmul(kv_ps[i][:, hs],
                                             kp8[:, c2, dc * P:(dc + 1) * P],
                                             v8[:, c2, hs],
                                             start=(chp == 0),
                                             stop=(chp == TOTCH // 2 - 1),
                                             perf_mode=DR)
                for i in range(2):
                    nc.scalar.activation(kv_sb[:, pass_ * 2 + i, :], kv_ps[i],
                                         AF.Copy, scale=1.0 / KS)

            # K_sum over all tokens (fp8 DR, from long-written kp8), then
            # transpose to per-partition column layout [P, DO]
            psks = ph.enter_context(tc.tile_pool(name="psks", bufs=1, space="PSUM"))
            ps_ks = [psks.tile([1, 512], F32, tag=f"ksr{h}", name=f"ksr{h}")
                     for h in range(2)]
            for chp in range(TOTCH // 2):
                c2 = slice(2 * chp, 2 * chp + 2)
                for h in range(2):
                    nc.tensor.matmul(ps_ks[h], ones8[:, :, 0:1],
                                     kp8[:, c2, h * 512:(h + 1) * 512],
                                     start=(chp == 0), stop=(chp == TOTCH // 2 - 1),
                                     perf_mode=DR)
            for h in range(2):
                hs = slice(h * 512, (h + 1) * 512)
                nc.scalar.activation(ksrow_sb[0:1, hs], ps_ks[h], AF.Copy,
                                     scale=1.0 / KS)
            ps_ksc = psks.tile([P, DO], F32, tag="kscol")
            for dc in range(DO):
                nc.tensor.matmul(ps_ksc[:, dc:dc + 1],
                                 ksrow_sb[0:1, dc * P:(dc + 1) * P],
                                 ones_one[0:1, 0:1], start=True, stop=True)
            nc.scalar.activation(ksum_sb[:, :, 0], ps_ksc, AF.Copy)

        # ---------------- Phase C: conv'' + local MLP + token mixer ----------------
        with ExitStack() as ph:
            io = ph.enter_context(tc.tile_pool(name="ioC", bufs=2))
            pipe = ph.enter_context(tc.tile_pool(name="pipeC", bufs=2))
            mid = ph.enter_context(tc.tile_pool(name="midC", bufs=1))
            sm = ph.enter_context(tc.tile_pool(name="smC", bufs=1))
            ps = ph.enter_context(tc.tile_pool(name="psC", bufs=2, space="PSUM"))
            pst = ph.enter_context(tc.tile_pool(name="pstC", bufs=1, space="PSUM"))

            def c_front(it):
                n0 = it * NT
                x_t = io.tile([P, DO, W], F16, tag="xC", name=f"x_{it}")
                lo, hi = n0 - 2, n0 + NT + 2
                if lo < 0:
                    nc.vector.memset(x_t[:, :, 0:2], 0.0)
                    nc.sync.dma_start(x_t[:, :, 2:W], xT[:, :, 0:hi])
                elif hi > N:
                    nc.vector.memset(x_t[:, :, W - 2:W], 0.0)
                    nc.sync.dma_start(x_t[:, :, 0:W - 2], xT[:, :, lo:N])
                else:
                    nc.sync.dma_start(x_t, xT[:, :, lo:hi])

                dcv = io.tile([P, DO, NT], F16, tag="dcvC", name=f"dcv_{it}")
                # diffusion dwconv'': center tap on ACT, side taps on DVE
                for o in range(DO):
                    nc.scalar.activation(dcv[:, o, :], x_t[:, o, 2:NT + 2],
                                         AF.Identity, bias=pp[:, o, CB:CB + 1],
                                         scale=pp[:, o, C1:C1 + 1])
                for o in range(DO):
                    nc.vector.scalar_tensor_tensor(dcv[:, o, :], x_t[:, o, 1:NT + 1],
                                                   pp[:, o, C0:C0 + 1], dcv[:, o, :],
                                                   OP.mult, OP.add)
                for o in range(DO):
                    nc.vector.scalar_tensor_tensor(dcv[:, o, :], x_t[:, o, 3:NT + 3],
                                                   pp[:, o, C2:C2 + 1], dcv[:, o, :],
                                                   OP.mult, OP.add)

                # local MLP first half
                h1_t = pipe.tile([P, DO, NT], F16, tag="h1", name=f"h1_{it}")
                for dc in range(DO):
                    ps_h = ps.tile([P, NT], F32, tag="psh1", name=f"psh1_{it}_{dc}")
                    for kc in range(DO):
                        nc.tensor.matmul(ps_h, w1_sb[:, kc, dc * P:(dc + 1) * P],
                                         x_t[:, kc, 2:NT + 2],
                                         start=(kc == 0), stop=(kc == DO - 1))
                    nc.scalar.activation(h1_t[:, dc, :], ps_h, AF.Gelu,
                                         bias=pp[:, dc, LUB1:LUB1 + 1])

                # token mixer LN stats
                sq_t = mid.tile([P, DO, W], F16, tag="tokA", name=f"sq_{it}")
                nc.scalar.activation(sq_t, x_t, AF.Square)
                ps_m = pst.tile([P, W], F32, tag="psm", name=f"psm_{it}")
                stats_mm(ps_m, onesD, x_t, W)
                ps_s = pst.tile([P, W], F32, tag="pss", name=f"pss_{it}")
                stats_mm(ps_s, onesD, sq_t, W)
                m_sb = sm.tile([P, W], F16, tag="msb", name=f"msb_{it}")
                nc.scalar.activation(m_sb, ps_m, AF.Copy)
                var = sm.tile([P, W], F32, tag="var", name=f"var_{it}")
                nc.scalar.activation(var, ps_m, AF.Square)
                nc.vector.tensor_sub(var, ps_s, var)
                nc.scalar.activation(var, var, AF.Sqrt, bias=eps_ln[:, 0:1])
                nc.vector.reciprocal_approx_fast(out=var, in_=var)
                rstd = sm.tile([P, W], F16, tag="rstd", name=f"rstd_{it}")
                nc.vector.tensor_copy(rstd, var)
                u_t = mid.tile([P, DO, W], F16, tag="tokA", name=f"u_{it}")
                for o in range(DO):
                    nc.vector.tensor_sub(u_t[:, o, :], x_t[:, o, :], m_sb)
                xm_t = mid.tile([P, DO, W], F16, tag="tokC", name=f"xm_{it}")
                for o in range(DO):
                    nc.vector.scalar_tensor_tensor(xm_t[:, o, :], u_t[:, o, :],
                                                   pp[:, o, TMG:TMG + 1], rstd,
                                                   OP.mult, OP.mult)
                if use_tmb:
                    for o in range(DO):
                        nc.vector.tensor_scalar_add(xm_t[:, o, :], xm_t[:, o, :],
                                                    pp[:, o, TMB:TMB + 1])
                # conv1: t_s[k] = conv1(xm)[k+1], k in [0, W-2)
                t_t = mid.tile([P, DO, W - 2], F16, tag="tokD", name=f"t_{it}")
                for o in range(DO):
                    nc.scalar.activation(t_t[:, o, :], xm_t[:, o, 1:W - 1],
                                         AF.Identity, bias=pp[:, o, TCB1:TCB1 + 1],
                                         scale=pp[:, o, T1:T1 + 1])
                for o in range(DO):
                    nc.vector.scalar_tensor_tensor(t_t[:, o, :], xm_t[:, o, 0:W - 2],
                                                   pp[:, o, T0:T0 + 1],
                                                   t_t[:, o, :], OP.mult, OP.add)
                for o in range(DO):
                    nc.vector.scalar_tensor_tensor(t_t[:, o, :], xm_t[:, o, 2:W],
                                                   pp[:, o, T2:T2 + 1],
                                                   t_t[:, o, :], OP.mult, OP.add)
                t2_t = pipe.tile([P, DO, W - 2], F16, tag="t2", name=f"t2_{it}")
                nc.scalar.activation(t2_t, t_t, AF.Gelu)
                if it == 0:
                    nc.vector.memset(t2_t[:, :, 0:1], 0.0)
                if it == NTILES - 1:
                    nc.vector.memset(t2_t[:, :, W - 3:W - 2], 0.0)
                return x_t, dcv, h1_t, t2_t

            def c_back(it, tiles):
                n0 = it * NT
                x_t, dcv, h1_t, t2_t = tiles
                acc = io.tile([P, DO, NT], F16, tag="accC", name=f"acc_{it}")
                for dc in range(DO):
                    ps_h = ps.tile([P, NT], F32, tag="psh2", name=f"psh2_{it}_{dc}")
                    for kc in range(DO):
                        nc.tensor.matmul(ps_h, w2_sb[:, kc, dc * P:(dc + 1) * P],
                                         h1_t[:, kc, :],
                                         start=(kc == 0), stop=False)
                    for tap in range(3):
                        nc.tensor.matmul(ps_h, diags[:, tap, dc, :],
                                         t2_t[:, dc, tap:NT + tap],
                                         start=False, stop=(tap == 2))
                    nc.vector.tensor_add(acc[:, dc, :], ps_h, dcv[:, dc, :])
                nc.sync.dma_start(acc_r[:, :, n0:n0 + NT], acc)

            pend = {0: c_front(0)}
            for it in range(NTILES):
                if it + 1 < NTILES:
                    pend[it + 1] = c_front(it + 1)
                c_back(it, pend.pop(it))

        wc_stack.close()

        # ---------------- Phase D: attention + LN1 + FFN + LN2 ----------------
        # Pipelined; numerator of tile t+1 is split into two half-groups that
        # are issued under tile t's two DVE-bound LN chains.
        with ExitStack() as ph:
            wpoolD = ph.enter_context(tc.tile_pool(name="wD", bufs=1))
            f1_sb = wpoolD.tile([P, DO, D], F8, tag="f1")
            nc.sync.dma_start(f1_sb, w8r["f18"])
            f2_sb = wpoolD.tile([P, DO, D], F8, tag="f2")
            nc.sync.dma_start(f2_sb, w8r["f28"])
            io = ph.enter_context(tc.tile_pool(name="ioD", bufs=3))
            mid = ph.enter_context(tc.tile_pool(name="midD", bufs=1))
            bk = ph.enter_context(tc.tile_pool(name="bkD", bufs=2))
            sm = ph.enter_context(tc.tile_pool(name="smD", bufs=2))
            ps = ph.enter_context(tc.tile_pool(name="psD", bufs=2, space="PSUM"))
            psf_pool = ph.enter_context(tc.tile_pool(name="psfD", bufs=2, space="PSUM"))
            pst = ph.enter_context(tc.tile_pool(name="pstD", bufs=1, space="PSUM"))

            def d_load(it):
                n0 = it * NT
                acc_t = io.tile([P, DO, NT], F16, tag="accD", name=f"accD_{it}")
                nc.sync.dma_start(acc_t, acc_r[:, :, n0:n0 + NT])
                return acc_t

            def d_front_a(it, acc_t):
                """norm row, 1/norm fold, numerator halves 0-3."""
                n0 = it * NT
                ps_n = ps.tile([P, NT], F32, tag="psnum", name=f"psn_{it}")
                for kc in range(DO):
                    nc.tensor.matmul(ps_n[0:1, :], ksum_sb[:, kc, :],
                                     qp[:, kc, n0:n0 + NT],
                                     start=(kc == 0), stop=(kc == DO - 1))
                nr = sm.tile([1, NT], F32, tag="nrD", name=f"nr_{it}")
                nc.vector.tensor_scalar_add(nr, ps_n[0:1, :], 1e-6)
                rr = sm.tile([1, NT], F32, tag="rrD", name=f"rr_{it}")
                nc.vector.reciprocal_approx_fast(out=rr, in_=nr)
                ps_rep = ps.tile([P, NT], F32, tag="psnum", name=f"psrep_{it}")
                nc.tensor.matmul(ps_rep, ones_1p_f32[0:1, :], rr, start=True,
                                 stop=True)
                rep_sb = mid.tile([P, NT], BF16, tag="repsb", name=f"rep_{it}")
                nc.scalar.activation(rep_sb, ps_rep, AF.Copy)
                for kc in range(DO):
                    nc.vector.tensor_mul(qp[:, kc, n0:n0 + NT],
                                         qp[:, kc, n0:n0 + NT], rep_sb)
                for ec in range(DO // 2):
                    ps_u = ps.tile([P, NT], F32, tag="psnum", name=f"psnum_{it}_{ec}")
                    for kc in range(DO):
                        nc.tensor.matmul(ps_u, kv_sb[:, kc, ec * P:(ec + 1) * P],
                                         qp[:, kc, n0:n0 + NT],
                                         start=(kc == 0), stop=(kc == DO - 1))
                    nc.vector.tensor_add(acc_t[:, ec, :], acc_t[:, ec, :], ps_u)
                return acc_t

            def d_front_b(it, acc_t):
                n0 = it * NT
                for ec in range(DO // 2, DO):
                    ps_u = ps.tile([P, NT], F32, tag="psnum", name=f"psnum_{it}_{ec}")
                    for kc in range(DO):
                        nc.tensor.matmul(ps_u, kv_sb[:, kc, ec * P:(ec + 1) * P],
                                         qp[:, kc, n0:n0 + NT],
                                         start=(kc == 0), stop=(kc == DO - 1))
                    nc.vector.tensor_add(acc_t[:, ec, :], acc_t[:, ec, :], ps_u)
                return acc_t

            def d_mid(it, acc_t):
                """LN1 stats + apply -> y1 (fp16)."""
                sq_t = mid.tile([P, DO, NT], F16, tag="sqD", name=f"sqD_{it}")
                nc.scalar.activation(sq_t, acc_t, AF.Square)
                ps_m1 = pst.tile([P, NT], F32, tag="psm1", name=f"psm1_{it}")
                stats_mm(ps_m1, onesD, acc_t, NT)
                ps_s1 = pst.tile([P, NT], F32, tag="pss1", name=f"pss1_{it}")
                stats_mm(ps_s1, onesD, sq_t, NT)
                m1_sb = sm.tile([P, NT], F16, tag="m1sb", name=f"m1_{it}")
                nc.scalar.activation(m1_sb, ps_m1, AF.Copy)
                var1 = sm.tile([P, NT], F32, tag="varD", name=f"var1_{it}")
                nc.scalar.activation(var1, ps_m1, AF.Square)
                nc.vector.tensor_sub(var1, ps_s1, var1)
                nc.scalar.activation(var1, var1, AF.Sqrt, bias=eps_ln[:, 0:1])
                nc.vector.reciprocal_approx_fast(out=var1, in_=var1)
                rstd1 = sm.tile([P, NT], F16, tag="rstdb", name=f"rstdb_{it}")
                nc.vector.tensor_copy(rstd1, var1)
                u1_t = mid.tile([P, DO, NT], F16, tag="u1", name=f"u1_{it}")
                for o in range(DO):
                    nc.vector.tensor_sub(u1_t[:, o, :], acc_t[:, o, :], m1_sb)
                y1_t = mid.tile([P, DO, NT], F16, tag="y1", name=f"y1_{it}")
                for o in range(DO):
                    nc.vector.scalar_tensor_tensor(y1_t[:, o, :], u1_t[:, o, :],
                                                   pp[:, o, N1G:N1G + 1], rstd1,
                                                   OP.mult, OP.mult)
                if use_n1b:
                    for o in range(DO):
                        nc.vector.tensor_scalar_add(y1_t[:, o, :], y1_t[:, o, :],
                                                    pp[:, o, N1B:N1B + 1])
                return y1_t

            def d_ffn(it, y1_t):
                # fp8 cast of y1 for the DR matmuls (residual stays fp16)
                y18_t = mid.tile([P, DO, NT], F8, tag="y18", name=f"y18_{it}")
                nc.scalar.activation(y18_t, y1_t, AF.Copy)
                f1h_t = mid.tile([P, DO, NT], F8, tag="f1h", name=f"f1h_{it}")
                for dc in range(DO):
                    ps_f = psf_pool.tile([P, NT], F32, tag="psf",
                                         name=f"psf1_{it}_{dc}")
                    for kcp in range(DO // 2):
                        ks2 = slice(2 * kcp, 2 * kcp + 2)
                        nc.tensor.matmul(ps_f, f1_sb[:, ks2, dc * P:(dc + 1) * P],
                                         y18_t[:, ks2, :],
                                         start=(kcp == 0), stop=(kcp == DO // 2 - 1),
                                         perf_mode=DR)
                    nc.scalar.activation(f1h_t[:, dc, :], ps_f, AF.Gelu,
                                         scale=1.0 / WS,
                                         bias=pp[:, dc, FFB1:FFB1 + 1])
                y2_t = bk.tile([P, DO, NT], F16, tag="y2", name=f"y2_{it}")
                for dc in range(DO):
                    ps_f = psf_pool.tile([P, NT], F32, tag="psf",
                                         name=f"psf2_{it}_{dc}")
                    for kcp in range(DO // 2):
                        ks2 = slice(2 * kcp, 2 * kcp + 2)
                        nc.tensor.matmul(ps_f, f2_sb[:, ks2, dc * P:(dc + 1) * P],
                                         f1h_t[:, ks2, :],
                                         start=(kcp == 0), stop=(kcp == DO // 2 - 1),
                                         perf_mode=DR)
                    nc.vector.scalar_tensor_tensor(y2_t[:, dc, :], ps_f,
                                                   1.0 / WS,
                                                   y1_t[:, dc, :], OP.mult, OP.add)
                    if use_fb2:
                        nc.scalar.activation(y2_t[:, dc, :], y2_t[:, dc, :],
                                             AF.Identity,
                                             bias=pp[:, dc, FFB2:FFB2 + 1])
                return y2_t

            def d_back(it, y2_t):
                n0 = it * NT
                sq2_t = bk.tile([P, DO, NT], F16, tag="sq2", name=f"sq2_{it}")
                nc.scalar.activation(sq2_t, y2_t, AF.Square)
                ps_m2 = pst.tile([P, NT], F32, tag="psm2", name=f"psm2_{it}")
                stats_mm(ps_m2, onesD, y2_t, NT)
                ps_s2 = pst.tile([P, NT], F32, tag="pss2", name=f"pss2_{it}")
                stats_mm(ps_s2, onesD, sq2_t, NT)
                m2_sb = sm.tile([P, NT], F32, tag="m2sb", name=f"m2_{it}")
                nc.scalar.activation(m2_sb, ps_m2, AF.Copy)
                var2 = sm.tile([P, NT], F32, tag="varD", name=f"var2_{it}")
                nc.scalar.activation(var2, ps_m2, AF.Square)
                nc.vector.tensor_sub(var2, ps_s2, var2)
                nc.scalar.activation(var2, var2, AF.Sqrt, bias=eps_ln[:, 0:1])
                nc.vector.reciprocal_approx_fast(out=var2, in_=var2)
                yo_t = bk.tile([P, DO, NT], F16, tag="yo", name=f"yo_{it}")
                for o in range(DO):
                    nc.vector.tensor_sub(yo_t[:, o, :], y2_t[:, o, :], m2_sb)
                for o in range(DO):
                    nc.vector.scalar_tensor_tensor(yo_t[:, o, :], yo_t[:, o, :],
                                                   pp[:, o, N2G:N2G + 1], var2,
                                                   OP.mult, OP.mult)
                if use_n2b:
                    for o in range(DO):
                        nc.vector.tensor_scalar_add(yo_t[:, o, :], yo_t[:, o, :],
                                                    pp[:, o, N2B:N2B + 1])
                nc.sync.dma_start(yT[:, :, n0:n0 + NT], yo_t)

            acc_tiles = {0: d_load(0)}
            if NTILES > 1:
                acc_tiles[1] = d_load(1)
            acc_cur = d_front_b(0, d_front_a(0, acc_tiles.pop(0)))
            acc_nxt = None
            y2_prev = None
            # d_back runs one slot late so its ACT/DVE chains overlap the
            # next tile's d_mid/d_ffn matmuls
            for it in range(NTILES):
                y1_cur = d_mid(it, acc_cur)
                if it + 2 < NTILES:
                    acc_tiles[it + 2] = d_load(it + 2)
                if it + 1 < NTILES:
                    acc_nxt = d_front_a(it + 1, acc_tiles.pop(it + 1))
                if y2_prev is not None:
                    d_back(it - 1, y2_prev)
                y2_cur = d_ffn(it, y1_cur)
                if it + 1 < NTILES:
                    acc_cur = d_front_b(it + 1, acc_nxt)
                y2_prev = y2_cur
            d_back(NTILES - 1, y2_prev)

    nc.compile()
    return nc


def make_in_maps(inputs, n_cores=8):
    """Host-side preprocessing: fold constants, transpose, cast, shard."""
    x = np.asarray(inputs["x"], np.float32)
    B, N, D_ = x.shape
    dt = float(np.asarray(inputs["delta_t"]))

    def g(k):
        return np.asarray(inputs[k], np.float32)

    diff_w, diff_b = g("diff_w"), g("diff_b")
    tm_w1, tm_cb1 = g("tm_w1"), g("tm_cb1")
    tm_w2, tm_cb2 = g("tm_w2"), g("tm_cb2")

    pp = np.zeros((P, DO, NPARAM), np.float32)

    def put(i, v):
        pp[:, :, i] = v.reshape(DO, P).T

    put(C0, dt * diff_w[:, 0, 0])
    put(C1, dt * diff_w[:, 0, 1] + (1.0 - dt))
    put(C2, dt * diff_w[:, 0, 2])
    put(CB, dt * diff_b + g("lu_b2") + tm_cb2)
    put(T0, tm_w1[:, 0, 0])
    put(T1, tm_w1[:, 0, 1])
    put(T2, tm_w1[:, 0, 2])
    put(TCB1, tm_cb1)
    put(U0, tm_w2[:, 0, 0])
    put(U1, tm_w2[:, 0, 1])
    put(U2, tm_w2[:, 0, 2])
    put(TMG, g("tm_g"))
    put(TMB, g("tm_beta"))
    put(N1G, g("n1_g"))
    put(N1B, g("n1_b"))
    put(N2G, g("n2_g"))
    put(N2B, g("n2_b"))
    put(LUB1, g("lu_b1"))
    put(FFB1, g("ff_b1"))
    put(FFB2, g("ff_b2"))

    diags = np.zeros((P, 3, DO, P), np.float32)
    idx = np.arange(P)
    for tap in range(3):
        for dc in range(DO):
            diags[idx, tap, dc, idx] = tm_w2[dc * P + idx, 0, tap]
    diags = diags.astype(F16_NP)

    rows = np.zeros((1, 3 * D), np.float32)
    rows[0, 0:D] = g("bq") * WS
    rows[0, D:2 * D] = g("bk") * KS
    rows[0, 2 * D:3 * D] = g("bv") * WS
    rows = rows.astype(BF16_NP)

    wt = {}
    for name, key in (("w1T", "lu_w1"), ("w2T", "lu_w2")):
        wt[name] = np.ascontiguousarray(g(key).T).astype(F16_NP)
    w8 = {}
    for name, key, s in (("wq8", "wq", WS), ("wk8", "wk", KS), ("wv8", "wv", WS),
                         ("f18", "ff_w1", WS), ("f28", "ff_w2", WS)):
        w8[name] = np.ascontiguousarray(g(key).T * s).astype(F8_NP)

    xT = np.ascontiguousarray(x.transpose(0, 2, 1)).astype(F16_NP)
    x8 = xT.astype(F8_NP)

    flags = dict(
        use_bq=bool(np.any(g("bq"))),
        use_bk=bool(np.any(g("bk"))),
        use_bv=bool(np.any(g("bv"))),
        use_tmb=bool(np.any(g("tm_beta"))),
        use_n1b=bool(np.any(g("n1_b"))),
        use_n2b=bool(np.any(g("n2_b"))),
        use_fb2=bool(np.any(g("ff_b2"))),
    )

    shared = {**wt, **w8, "pp": pp, "rows": rows, "diags": diags}
    in_maps = [{**shared, "x_T": xT[b], "x_8": x8[b]} for b in range(B)]
    return in_maps, flags, (B, N)


_NC_CACHE = {}


def kernel(**inputs):
    in_maps, flags, (B, N) = make_in_maps(inputs)
    key = (N, tuple(sorted(flags.items())))
    if key not in _NC_CACHE:
        _NC_CACHE[key] = build_nc(N=N, NT=512, **flags)
    nc = _NC_CACHE[key]
    res = run_bass_kernel_spmd(nc, in_maps, list(range(B)))
    y = np.stack([res.results[b]["y_T"] for b in range(B)])
    return np.ascontiguousarray(y.transpose(0, 2, 1)).astype(np.float32)


# revision 54
# speedup vs baseline: 1.1854x; 1.0633x over previous
"""Trainium2 Bass kernel for nn_DiffuRNNLayer (B=8, N=2048, D=1024).

Sharding: data-parallel over batch - one batch element per NeuronCore (8 cores).
Per-core phases:
  A: Q/K/V projections in fp8 DoubleRow (+elu+1); Qp (bf16, scaled by WS),
     Kp (fp8, x16) / V (fp8) SBUF-resident; K_sum accumulated inline.
  B: KV = Kp^T V via fp8 DoubleRow from SBUF-resident Kp/V.
  C: acc = dwconv''(x) + MLP(x) + tokenmixer(LN(x)) in fp16; spill acc.
  D: attn numerator with 1/norm folded into Qp, acc += attn; LN1; FFN
     residual; LN2; write y^T (fp16).
"""

import numpy as np
import ml_dtypes
from contextlib import ExitStack

import concourse.bass as bass
import concourse.bacc as bacc
import concourse.tile as tile
import concourse.mybir as mybir
from concourse.bass_utils import run_bass_kernel_spmd

F32 = mybir.dt.float32
BF16 = mybir.dt.bfloat16
F16 = mybir.dt.float16
F8 = mybir.dt.float8e4
AF = mybir.ActivationFunctionType
OP = mybir.AluOpType
DR = mybir.MatmulPerfMode.DoubleRow
BF16_NP = ml_dtypes.bfloat16
F16_NP = np.float16
F8_NP = ml_dtypes.float8_e4m3

P = 128
D = 1024
DO = D // P  # 8 chunks of the channel dim
WS = 256.0   # fp8 weight scale for wq/wv
KS = 16.0    # fp8 weight scale for wk (kp8 = KS*Kp must stay under 240)
LNWS = float(np.log(WS))
LNKS = float(np.log(KS))

# pp param-plane indices (per-partition params, laid out [128, DO, NP])
(C0, C1, C2, CB, T0, T1, T2, TCB1, U0, U1, U2,
 TMG, TMB, N1G, N1B, N2G, N2B, LUB1, FFB1, FFB2) = range(20)
NPARAM = 20


def build_nc(N=2048, NT=512, use_bq=False, use_bk=False, use_bv=False,
             use_tmb=False, use_n1b=False, use_n2b=False, use_fb2=False,
             debug=False):
    NTILES = N // NT
    NTA = 1024             # phase-A supertile width
    NST = N // NTA
    NCH_A = NTA // P       # 128-token chunks per supertile
    TOTCH = N // P
    W = NT + 4             # phase-C tile width with +-2 halo
    assert N % NT == 0 and NT % P == 0 and N % NTA == 0

    nc = bacc.Bacc(None, target_bir_lowering=False, debug=debug)

    xT_d = nc.dram_tensor("x_T", [D, N], F16, kind="ExternalInput")
    x8_d = nc.dram_tensor("x_8", [D, N], F8, kind="ExternalInput")
    w8_d = {}
    for name in ("wq8", "wk8", "wv8"):
        w8_d[name] = nc.dram_tensor(name, [D, D], F8, kind="ExternalInput")
    w_d = {}
    for name in ("w1T", "w2T"):
        w_d[name] = nc.dram_tensor(name, [D, D], F16, kind="ExternalInput")
    for name in ("f18", "f28"):
        w8_d[name] = nc.dram_tensor(name, [D, D], F8, kind="ExternalInput")
    pp_d = nc.dram_tensor("pp", [P, DO, NPARAM], F32, kind="ExternalInput")
    diags_d = nc.dram_tensor("diags", [P, 3, DO, P], F16, kind="ExternalInput")
    rows_d = nc.dram_tensor("rows", [1, 3 * D], BF16, kind="ExternalInput")
    yT_d = nc.dram_tensor("y_T", [D, N], F16, kind="ExternalOutput")

    acc_sp = nc.dram_tensor("acc_sp", [D, N], F16)

    xT = xT_d.rearrange("(o p) n -> p o n", p=P)
    x8r = x8_d.rearrange("(o p) n -> p o n", p=P)
    w8r = {k: v.rearrange("(o p) n -> p o n", p=P) for k, v in w8_d.items()}
    wr = {k: v.rearrange("(o p) n -> p o n", p=P) for k, v in w_d.items()}
    acc_r = acc_sp.rearrange("(o p) n -> p o n", p=P)
    yT = yT_d.rearrange("(o p) n -> p o n", p=P)

    with tile.TileContext(nc) as tc, ExitStack() as top:
        persist = top.enter_context(tc.tile_pool(name="persist", bufs=1))
        ones_1p_f32 = persist.tile([1, P], F32)
        nc.vector.memset(ones_1p_f32, 1.0)
        ones_one = persist.tile([1, 1], BF16)
        nc.vector.memset(ones_one, 1.0)
        ones8 = persist.tile([P, 2, 16], F8)
        nc.vector.memset(ones8, 1.0)
        ksrow_sb = persist.tile([1, D], BF16)
        onesD = persist.tile([P, P], F16)
        nc.vector.memset(onesD, 1.0 / D)
        eps_ln = persist.tile([P, 1], F32)
        nc.vector.memset(eps_ln, 1e-5)
        lnws = persist.tile([P, 1], F32)
        nc.vector.memset(lnws, LNWS)
        lnks = persist.tile([P, 1], F32)
        nc.vector.memset(lnks, LNKS)
        kv_sb = persist.tile([P, DO, D], BF16)
        ksum_sb = persist.tile([P, DO, 1], BF16)
        qp = persist.tile([P, DO, N], BF16)  # WS-scaled Qp, resident
        pp = persist.tile([P, DO, NPARAM], F32)
        diags = persist.tile([P, 3, DO, P], F16)
        rows = ones_row = ones_1p_bf = None
        if use_bq or use_bk or use_bv:
            rows = persist.tile([1, 3 * D], BF16)
            ones_row = persist.tile([1, NTA], BF16)
            nc.vector.memset(ones_row, 1.0)
            ones_1p_bf = persist.tile([1, P], BF16)
            nc.vector.memset(ones_1p_bf, 1.0)

        def stats_mm(psum, lhs_ones, rhs3, width):
            """Accumulate over DO k-chunks: psum[:, j] = mean over channel dim,
            replicated across partitions.  rhs3: [P, DO, width]."""
            for c0 in range(0, width, 512):
                cw = min(512, width - c0)
                for kc in range(DO):
                    nc.tensor.matmul(psum[:, c0:c0 + cw], lhs_ones,
                                     rhs3[:, kc, c0:c0 + cw],
                                     start=(kc == 0), stop=(kc == DO - 1))

        # Phase-C weight pool created ahead of the AB stack (LIFO pool order);
        # the DMAs are issued after phase A's loop so they don't compete with
        # A's critical loads.
        wc_stack = top.enter_context(ExitStack())
        wpoolC = wc_stack.enter_context(tc.tile_pool(name="wC", bufs=1))
        w1_sb = wpoolC.tile([P, DO, D], F16, tag="w1")
        w2_sb = wpoolC.tile([P, DO, D], F16, tag="w2")
        x0_t = wpoolC.tile([P, DO, W], F16, tag="x0")

        # ---------------- Phases A+B: QKV + KV (fp8 DoubleRow) ----------------
        with ExitStack() as ph:
            wpool = ph.enter_context(tc.tile_pool(name="wA", bufs=1))
            wq_sb = wpool.tile([P, DO, D], F8, tag="wq")
            nc.sync.dma_start(wq_sb, w8r["wq8"])
            kvres = ph.enter_context(tc.tile_pool(name="kvres", bufs=1))
            kp8 = kvres.tile([P, TOTCH, D], F8, tag="kp8")
            v8 = kvres.tile([P, TOTCH, D], F8, tag="v8")
            io = ph.enter_context(tc.tile_pool(name="ioA", bufs=2))
            x8_0 = io.tile([P, DO, NTA], F8, tag="x8A", name="x8_0")
            nc.sync.dma_start(x8_0, x8r[:, :, 0:NTA])
            wk_sb = wpool.tile([P, DO, D], F8, tag="wk")
            nc.sync.dma_start(wk_sb, w8r["wk8"])
            wv_sb = wpool.tile([P, DO, D], F8, tag="wv")
            nc.sync.dma_start(wv_sb, w8r["wv8"])
            if use_bq or use_bk or use_bv:
                nc.sync.dma_start(rows, rows_d[:])
            nc.sync.dma_start(pp, pp_d[:])
            nc.sync.dma_start(diags, diags_d[:])

            ev = ph.enter_context(tc.tile_pool(name="evA", bufs=4))
            pa = ph.enter_context(ExitStack())
            psQ = pa.enter_context(tc.tile_pool(name="psQA", bufs=2, space="PSUM"))
            psK = pa.enter_context(tc.tile_pool(name="psKA", bufs=2, space="PSUM"))
            psV = pa.enter_context(tc.tile_pool(name="psVA", bufs=2, space="PSUM"))

            def do_q(st, x8_t, n0):
                for dc in range(DO):
                    ps_q = psQ.tile([P, NTA], F32, tag="psq", name=f"psq{st}_{dc}")
                    for kcp in range(DO // 2):
                        ks2 = slice(2 * kcp, 2 * kcp + 2)
                        for h in range(2):
                            hs = slice(h * 512, (h + 1) * 512)
                            nc.tensor.matmul(ps_q[:, hs],
                                             wq_sb[:, ks2, dc * P:(dc + 1) * P],
                                             x8_t[:, ks2, hs],
                                             start=(kcp == 0),
                                             stop=(kcp == DO // 2 - 1 and not use_bq),
                                             perf_mode=DR)
                    if use_bq:
                        for h in range(2):
                            hs = slice(h * 512, (h + 1) * 512)
                            nc.tensor.matmul(ps_q[:, hs],
                                             rows[0:1, dc * P:(dc + 1) * P],
                                             ones_row[0:1, hs], start=False,
                                             stop=True)
                    # e_all = WS*exp(q); e1 = min(e_all, WS) = WS*exp(min(q,0))
                    e_all = ev.tile([P, NTA], BF16, tag="eQ")
                    nc.scalar.activation(e_all, ps_q, AF.Exp,
                                         scale=1.0 / WS, bias=lnws[:, 0:1])
                    e1 = ev.tile([P, NTA], BF16, tag="e1Q")
                    nc.vector.tensor_scalar_min(e1, e_all, float(WS))
                    # qp' = max(WS*q, 0) + WS*exp(min(q,0)) = WS*Qp
                    nc.vector.scalar_tensor_tensor(
                        qp[:, dc, n0:n0 + NTA],
                        ps_q, 0.0, e1, OP.max, OP.add)

            def do_kv(st, x8_t, n0):
                for ch in range(NCH_A):
                    cs = slice(ch * P, (ch + 1) * P)
                    chg = st * NCH_A + ch
                    for h in range(2):
                        hs = slice(h * 512, (h + 1) * 512)
                        ps_k = psK.tile([P, 512], F32, tag="psk")
                        ps_v = psV.tile([P, 512], F32, tag="psv")
                        for kcp in range(DO // 2):
                            ks2 = slice(2 * kcp, 2 * kcp + 2)
                            nc.tensor.matmul(ps_k, x8_t[:, ks2, cs],
                                             wk_sb[:, ks2, hs],
                                             start=(kcp == 0),
                                             stop=(kcp == DO // 2 - 1 and not use_bk),
                                             perf_mode=DR)
                            nc.tensor.matmul(ps_v, x8_t[:, ks2, cs],
                                             wv_sb[:, ks2, hs],
                                             start=(kcp == 0),
                                             stop=(kcp == DO // 2 - 1 and not use_bv),
                                             perf_mode=DR)
                        if use_bk:
                            nc.tensor.matmul(ps_k, ones_1p_bf[0:1, :],
                                             rows[0:1, D + h * 512:D + (h + 1) * 512],
                                             start=False, stop=True)
                        if use_bv:
                            nc.tensor.matmul(ps_v, ones_1p_bf[0:1, :],
                                             rows[0:1, 2 * D + h * 512:2 * D + (h + 1) * 512],
                                             start=False, stop=True)
                        # kp8 = KS*Kp = max(KS*k, 0) + min(KS*e^k, KS)
                        ek = ev.tile([P, 512], BF16, tag="eK")
                        nc.scalar.activation(ek, ps_k, AF.Exp,
                                             scale=1.0 / KS, bias=lnks[:, 0:1])
                        e1k = ev.tile([P, 512], BF16, tag="e1K")
                        nc.vector.tensor_scalar_min(e1k, ek, float(KS))
                        nc.vector.scalar_tensor_tensor(kp8[:, chg, hs], ps_k, 0.0,
                                                       e1k, OP.max, OP.add)
                        nc.scalar.activation(v8[:, chg, hs], ps_v, AF.Copy,
                                             scale=1.0 / WS)


            for st in range(NST):
                n0 = st * NTA
                if st == 0:
                    x8_t = x8_0
                else:
                    x8_t = io.tile([P, DO, NTA], F8, tag="x8A", name=f"x8_{st}")
                    nc.sync.dma_start(x8_t, x8r[:, :, n0:n0 + NTA])
                if st < NST - 1:
                    do_q(st, x8_t, n0)
                    do_kv(st, x8_t, n0)
                else:
                    # last supertile: K/V first so phase B's inputs finish early
                    do_kv(st, x8_t, n0)
                    do_q(st, x8_t, n0)

            # prefetch phase-C weights and first x tile while A/B compute
            nc.sync.dma_start(w1_sb, wr["w1T"])
            nc.sync.dma_start(w2_sb, wr["w2T"])
            nc.vector.memset(x0_t[:, :, 0:2], 0.0)
            nc.sync.dma_start(x0_t[:, :, 2:W], xT[:, :, 0:NT + 2])

            pa.close()

            # ---------------- Phase B: KV accumulation (fp8 DR) ----------------
            psB = ph.enter_context(tc.tile_pool(name="psB", bufs=1, space="PSUM"))
            for pass_ in range(4):
                kv_ps = [psB.tile([P, NTA], F32, tag=f"kvps{i}", name=f"kvps{pass_}_{i}")
                         for i in range(2)]
                for chp in range(TOTCH // 2):
                    c2 = slice(2 * chp, 2 * chp + 2)
                    for i in range(2):
                        dc = pass_ * 2 + i
                        for h in range(2):
                            hs = slice(h * 512, (h + 1) * 512)
                            nc.tensor.matmul(kv_ps[i][:, hs],
                                             kp8[:, c2, dc * P:(dc + 1) * P],
                                             v8[:, c2, hs],
                                             start=(chp == 0),
                                             stop=(chp == TOTCH // 2 - 1),
                                             perf_mode=DR)
                for i in range(2):
                    nc.scalar.activation(kv_sb[:, pass_ * 2 + i, :], kv_ps[i],
                                         AF.Copy, scale=1.0 / KS)

            # K_sum over all tokens (fp8 DR, from long-written kp8), then
            # transpose to per-partition column layout [P, DO]
            psks = ph.enter_context(tc.tile_pool(name="psks", bufs=1, space="PSUM"))
            ps_ks = [psks.tile([1, 512], F32, tag=f"ksr{h}", name=f"ksr{h}")
                     for h in range(2)]
            for chp in range(TOTCH // 2):
                c2 = slice(2 * chp, 2 * chp + 2)
                for h in range(2):
                    nc.tensor.matmul(ps_ks[h], ones8[:, :, 0:1],
                                     kp8[:, c2, h * 512:(h + 1) * 512],
                                     start=(chp == 0), stop=(chp == TOTCH // 2 - 1),
                                     perf_mode=DR)
            for h in range(2):
                hs = slice(h * 512, (h + 1) * 512)
                nc.scalar.activation(ksrow_sb[0:1, hs], ps_ks[h], AF.Copy,
                                     scale=1.0 / KS)
            ps_ksc = psks.tile([P, DO], F32, tag="kscol")
            for dc in range(DO):
                nc.tensor.matmul(ps_ksc[:, dc:dc + 1],
                                 ksrow_sb[0:1, dc * P:(dc + 1) * P],
                                 ones_one[0:1, 0:1], start=True, stop=True)
            nc.scalar.activation(ksum_sb[:, :, 0], ps_ksc, AF.Copy)

        # ---------------- Phase C: conv'' + local MLP + token mixer ----------------
        with ExitStack() as ph:
            io = ph.enter_context(tc.tile_pool(name="ioC", bufs=2))
            pipe = ph.enter_context(tc.tile_pool(name="pipeC", bufs=2))
            mid = ph.enter_context(tc.tile_pool(name="midC", bufs=1))
            sm = ph.enter_context(tc.tile_pool(name="smC", bufs=1))
            ps = ph.enter_context(tc.tile_pool(name="psC", bufs=2, space="PSUM"))
            pst = ph.enter_context(tc.tile_pool(name="pstC", bufs=1, space="PSUM"))

            def c_front(it):
                n0 = it * NT
                lo, hi = n0 - 2, n0 + NT + 2
                if it == 0:
                    x_t = x0_t  # preloaded during phases A/B
                else:
                    x_t = io.tile([P, DO, W], F16, tag="xC", name=f"x_{it}")
                    if hi > N:
                        nc.vector.memset(x_t[:, :, W - 2:W], 0.0)
                        nc.sync.dma_start(x_t[:, :, 0:W - 2], xT[:, :, lo:N])
                    else:
                        nc.sync.dma_start(x_t, xT[:, :, lo:hi])

                dcv = io.tile([P, DO, NT], F16, tag="dcvC", name=f"dcv_{it}")
                # diffusion dwconv'': center tap on ACT, side taps on DVE
                for o in range(DO):
                    nc.scalar.activation(dcv[:, o, :], x_t[:, o, 2:NT + 2],
                                         AF.Identity, bias=pp[:, o, CB:CB + 1],
                                         scale=pp[:, o, C1:C1 + 1])
                for o in range(DO):
                    nc.vector.scalar_tensor_tensor(dcv[:, o, :], x_t[:, o, 1:NT + 1],
                                                   pp[:, o, C0:C0 + 1], dcv[:, o, :],
                                                   OP.mult, OP.add)
                for o in range(DO):
                    nc.vector.scalar_tensor_tensor(dcv[:, o, :], x_t[:, o, 3:NT + 3],
                                                   pp[:, o, C2:C2 + 1], dcv[:, o, :],
                                                   OP.mult, OP.add)

                # local MLP first half
                h1_t = pipe.tile([P, DO, NT], F16, tag="h1", name=f"h1_{it}")
                for dc in range(DO):
                    ps_h = ps.tile([P, NT], F32, tag="psh1", name=f"psh1_{it}_{dc}")
                    for kc in range(DO):
                        nc.tensor.matmul(ps_h, w1_sb[:, kc, dc * P:(dc + 1) * P],
                                         x_t[:, kc, 2:NT + 2],
                                         start=(kc == 0), stop=(kc == DO - 1))
                    nc.scalar.activation(h1_t[:, dc, :], ps_h, AF.Gelu,
                                         bias=pp[:, dc, LUB1:LUB1 + 1])

                # token mixer LN stats
                sq_t = mid.tile([P, DO, W], F16, tag="tokA", name=f"sq_{it}")
                nc.scalar.activation(sq_t, x_t, AF.Square)
                ps_m = pst.tile([P, W], F32, tag="psm", name=f"psm_{it}")
                stats_mm(ps_m, onesD, x_t, W)
                ps_s = pst.tile([P, W], F32, tag="pss", name=f"pss_{it}")
                stats_mm(ps_s, onesD, sq_t, W)
                m_sb = sm.tile([P, W], F16, tag="msb", name=f"msb_{it}")
                nc.scalar.activation(m_sb, ps_m, AF.Copy)
                var = sm.tile([P, W], F32, tag="var", name=f"var_{it}")
                nc.scalar.activation(var, ps_m, AF.Square)
                nc.vector.tensor_sub(var, ps_s, var)
                nc.scalar.activation(var, var, AF.Sqrt, bias=eps_ln[:, 0:1])
                nc.vector.reciprocal_approx_fast(out=var, in_=var)
                rstd = sm.tile([P, W], F16, tag="rstd", name=f"rstd_{it}")
                nc.vector.tensor_copy(rstd, var)
                u_t = mid.tile([P, DO, W], F16, tag="tokA", name=f"u_{it}")
                for o in range(DO):
                    nc.vector.tensor_sub(u_t[:, o, :], x_t[:, o, :], m_sb)
                xm_t = mid.tile([P, DO, W], F16, tag="tokC", name=f"xm_{it}")
                for o in range(DO):
                    nc.vector.scalar_tensor_tensor(xm_t[:, o, :], u_t[:, o, :],
                                                   pp[:, o, TMG:TMG + 1], rstd,
                                                   OP.mult, OP.mult)
                if use_tmb:
                    for o in range(DO):
                        nc.vector.tensor_scalar_add(xm_t[:, o, :], xm_t[:, o, :],
                                                    pp[:, o, TMB:TMB + 1])
                # conv1: t_s[k] = conv1(xm)[k+1], k in [0, W-2)
                t_t = mid.tile([P, DO, W - 2], F16, tag="tokD", name=f"t_{it}")
                for o in range(DO):
                    nc.scalar.activation(t_t[:, o, :], xm_t[:, o, 1:W - 1],
                                         AF.Identity, bias=pp[:, o, TCB1:TCB1 + 1],
                                         scale=pp[:, o, T1:T1 + 1])
                for o in range(DO):
                    nc.vector.scalar_tensor_tensor(t_t[:, o, :], xm_t[:, o, 0:W - 2],
                                                   pp[:, o, T0:T0 + 1],
                                                   t_t[:, o, :], OP.mult, OP.add)
                for o in range(DO):
                    nc.vector.scalar_tensor_tensor(t_t[:, o, :], xm_t[:, o, 2:W],
                                                   pp[:, o, T2:T2 + 1],
                                                   t_t[:, o, :], OP.mult, OP.add)
                t2_t = pipe.tile([P, DO, W - 2], F16, tag="t2", name=f"t2_{it}")
                nc.scalar.activation(t2_t, t_t, AF.Gelu)
                if it == 0:
                    nc.vector.memset(t2_t[:, :, 0:1], 0.0)
                if it == NTILES - 1:
                    nc.vector.memset(t2_t[:, :, W - 3:W - 2], 0.0)
                return x_t, dcv, h1_t, t2_t

            def c_back(it, tiles):
                n0 = it * NT
                x_t, dcv, h1_t, t2_t = tiles
                acc = io.tile([P, DO, NT], F16, tag="accC", name=f"acc_{it}")
                for dc in range(DO):
                    ps_h = ps.tile([P, NT], F32, tag="psh2", name=f"psh2_{it}_{dc}")
                    for kc in range(DO):
                        nc.tensor.matmul(ps_h, w2_sb[:, kc, dc * P:(dc + 1) * P],
                                         h1_t[:, kc, :],
                                         start=(kc == 0), stop=False)
                    for tap in range(3):
                        nc.tensor.matmul(ps_h, diags[:, tap, dc, :],
                                         t2_t[:, dc, tap:NT + tap],
                                         start=False, stop=(tap == 2))
                    nc.vector.tensor_add(acc[:, dc, :], ps_h, dcv[:, dc, :])
                nc.sync.dma_start(acc_r[:, :, n0:n0 + NT], acc)

            pend = {0: c_front(0)}
            for it in range(NTILES):
                if it + 1 < NTILES:
                    pend[it + 1] = c_front(it + 1)
                c_back(it, pend.pop(it))

        wc_stack.close()

        # ---------------- Phase D: attention + LN1 + FFN + LN2 ----------------
        # Pipelined; numerator of tile t+1 is split into two half-groups that
        # are issued under tile t's two DVE-bound LN chains.
        with ExitStack() as ph:
            wpoolD = ph.enter_context(tc.tile_pool(name="wD", bufs=1))
            f1_sb = wpoolD.tile([P, DO, D], F8, tag="f1")
            nc.sync.dma_start(f1_sb, w8r["f18"])
            f2_sb = wpoolD.tile([P, DO, D], F8, tag="f2")
            nc.sync.dma_start(f2_sb, w8r["f28"])
            io = ph.enter_context(tc.tile_pool(name="ioD", bufs=3))
            mid = ph.enter_context(tc.tile_pool(name="midD", bufs=1))
            bk = ph.enter_context(tc.tile_pool(name="bkD", bufs=2))
            sm = ph.enter_context(tc.tile_pool(name="smD", bufs=2))
            ps = ph.enter_context(tc.tile_pool(name="psD", bufs=2, space="PSUM"))
            psf_pool = ph.enter_context(tc.tile_pool(name="psfD", bufs=2, space="PSUM"))
            pst = ph.enter_context(tc.tile_pool(name="pstD", bufs=1, space="PSUM"))

            def d_load(it):
                n0 = it * NT
                acc_t = io.tile([P, DO, NT], F16, tag="accD", name=f"accD_{it}")
                nc.sync.dma_start(acc_t, acc_r[:, :, n0:n0 + NT])
                return acc_t

            def d_front_a(it, acc_t):
                """norm row, 1/norm fold, numerator halves 0-3."""
                n0 = it * NT
                ps_n = ps.tile([P, NT], F32, tag="psnum", name=f"psn_{it}")
                for kc in range(DO):
                    nc.tensor.matmul(ps_n[0:1, :], ksum_sb[:, kc, :],
                                     qp[:, kc, n0:n0 + NT],
                                     start=(kc == 0), stop=(kc == DO - 1))
                nr = sm.tile([1, NT], F32, tag="nrD", name=f"nr_{it}")
                nc.vector.tensor_scalar_add(nr, ps_n[0:1, :], 1e-6)
                rr = sm.tile([1, NT], F32, tag="rrD", name=f"rr_{it}")
                nc.vector.reciprocal_approx_fast(out=rr, in_=nr)
                ps_rep = ps.tile([P, NT], F32, tag="psnum", name=f"psrep_{it}")
                nc.tensor.matmul(ps_rep, ones_1p_f32[0:1, :], rr, start=True,
                                 stop=True)
                rep_sb = mid.tile([P, NT], BF16, tag="repsb", name=f"rep_{it}")
                nc.scalar.activation(rep_sb, ps_rep, AF.Copy)
                for kc in range(DO):
                    nc.vector.tensor_mul(qp[:, kc, n0:n0 + NT],
                                         qp[:, kc, n0:n0 + NT], rep_sb)
                for ec in range(DO // 2):
                    ps_u = ps.tile([P, NT], F32, tag="psnum", name=f"psnum_{it}_{ec}")
                    for kc in range(DO):
                        nc.tensor.matmul(ps_u, kv_sb[:, kc, ec * P:(ec + 1) * P],
                                         qp[:, kc, n0:n0 + NT],
                                         start=(kc == 0), stop=(kc == DO - 1))
                    nc.vector.tensor_add(acc_t[:, ec, :], acc_t[:, ec, :], ps_u)
                return acc_t

            def d_front_b(it, acc_t):
                n0 = it * NT
                for ec in range(DO // 2, DO):
                    ps_u = ps.tile([P, NT], F32, tag="psnum", name=f"psnum_{it}_{ec}")
                    for kc in range(DO):
                        nc.tensor.matmul(ps_u, kv_sb[:, kc, ec * P:(ec + 1) * P],
                                         qp[:, kc, n0:n0 + NT],
                                         start=(kc == 0), stop=(kc == DO - 1))
                    nc.vector.tensor_add(acc_t[:, ec, :], acc_t[:, ec, :], ps_u)
                return acc_t

            def d_mid(it, acc_t):
                """LN1 stats + apply -> y1 (fp16)."""
                sq_t = mid.tile([P, DO, NT], F16, tag="sqD", name=f"sqD_{it}")
                nc.scalar.activation(sq_t, acc_t, AF.Square)
                ps_m1 = pst.tile([P, NT], F32, tag="psm1", name=f"psm1_{it}")
                stats_mm(ps_m1, onesD, acc_t, NT)
                ps_s1 = pst.tile([P, NT], F32, tag="pss1", name=f"pss1_{it}")
                stats_mm(ps_s1, onesD, sq_t, NT)
                m1_sb = sm.tile([P, NT], F16, tag="m1sb", name=f"m1_{it}")
                nc.scalar.activation(m1_sb, ps_m1, AF.Copy)
                var1 = sm.tile([P, NT], F32, tag="varD", name=f"var1_{it}")
                nc.scalar.activation(var1, ps_m1, AF.Square)
                nc.vector.tensor_sub(var1, ps_s1, var1)
                nc.scalar.activation(var1, var1, AF.Sqrt, bias=eps_ln[:, 0:1])
                nc.vector.reciprocal_approx_fast(out=var1, in_=var1)
                rstd1 = sm.tile([P, NT], F16, tag="rstdb", name=f"rstdb_{it}")
                nc.vector.tensor_copy(rstd1, var1)
                u1_t = mid.tile([P, DO, NT], F16, tag="u1", name=f"u1_{it}")
                for o in range(DO):
                    nc.vector.tensor_sub(u1_t[:, o, :], acc_t[:, o, :], m1_sb)
                y1_t = mid.tile([P, DO, NT], F16, tag="y1", name=f"y1_{it}")
                for o in range(DO):
                    nc.vector.scalar_tensor_tensor(y1_t[:, o, :], u1_t[:, o, :],
                                                   pp[:, o, N1G:N1G + 1], rstd1,
                                                   OP.mult, OP.mult)
                if use_n1b:
                    for o in range(DO):
                        nc.vector.tensor_scalar_add(y1_t[:, o, :], y1_t[:, o, :],
                                                    pp[:, o, N1B:N1B + 1])
                return y1_t

            def d_ffn(it, y1_t):
                # fp8 cast of y1 for the DR matmuls (residual stays fp16)
                y18_t = mid.tile([P, DO, NT], F8, tag="y18", name=f"y18_{it}")
                nc.scalar.activation(y18_t, y1_t, AF.Copy)
                f1h_t = mid.tile([P, DO, NT], F8, tag="f1h", name=f"f1h_{it}")
                for dc in range(DO):
                    ps_f = psf_pool.tile([P, NT], F32, tag="psf",
                                         name=f"psf1_{it}_{dc}")
                    for kcp in range(DO // 2):
                        ks2 = slice(2 * kcp, 2 * kcp + 2)
                        nc.tensor.matmul(ps_f, f1_sb[:, ks2, dc * P:(dc + 1) * P],
                                         y18_t[:, ks2, :],
                                         start=(kcp == 0), stop=(kcp == DO // 2 - 1),
                                         perf_mode=DR)
                    nc.scalar.activation(f1h_t[:, dc, :], ps_f, AF.Gelu,
                                         scale=1.0 / WS,
                                         bias=pp[:, dc, FFB1:FFB1 + 1])
                y2_t = bk.tile([P, DO, NT], F16, tag="y2", name=f"y2_{it}")
                for dc in range(DO):
                    ps_f = psf_pool.tile([P, NT], F32, tag="psf",
                                         name=f"psf2_{it}_{dc}")
                    for kcp in range(DO // 2):
                        ks2 = slice(2 * kcp, 2 * kcp + 2)
                        nc.tensor.matmul(ps_f, f2_sb[:, ks2, dc * P:(dc + 1) * P],
                                         f1h_t[:, ks2, :],
                                         start=(kcp == 0), stop=(kcp == DO // 2 - 1),
                                         perf_mode=DR)
                    nc.vector.scalar_tensor_tensor(y2_t[:, dc, :], ps_f,
                                                   1.0 / WS,
                                                   y1_t[:, dc, :], OP.mult, OP.add)
                    if use_fb2:
                        nc.scalar.activation(y2_t[:, dc, :], y2_t[:, dc, :],
                                             AF.Identity,
                                             bias=pp[:, dc, FFB2:FFB2 + 1])
                return y2_t

            def d_back(it, y2_t, nsplit=1):
                n0 = it * NT
                sq2_t = bk.tile([P, DO, NT], F16, tag="sq2", name=f"sq2_{it}")
                nc.scalar.activation(sq2_t, y2_t, AF.Square)
                HW_ = NT // nsplit
                for sp in range(nsplit):
                    cs = slice(sp * HW_, (sp + 1) * HW_)
                    ps_m2 = pst.tile([P, HW_], F32, tag="psm2",
                                     name=f"psm2_{it}_{sp}")
                    for kc in range(DO):
                        nc.tensor.matmul(ps_m2, onesD, y2_t[:, kc, cs],
                                         start=(kc == 0), stop=(kc == DO - 1))
                    ps_s2 = pst.tile([P, HW_], F32, tag="pss2",
                                     name=f"pss2_{it}_{sp}")
                    for kc in range(DO):
                        nc.tensor.matmul(ps_s2, onesD, sq2_t[:, kc, cs],
                                         start=(kc == 0), stop=(kc == DO - 1))
                    m2_sb = sm.tile([P, HW_], F32, tag="m2sb", name=f"m2_{it}_{sp}")
                    nc.scalar.activation(m2_sb, ps_m2, AF.Copy)
                    var2 = sm.tile([P, HW_], F32, tag="varD", name=f"var2_{it}_{sp}")
                    nc.scalar.activation(var2, ps_m2, AF.Square)
                    nc.vector.tensor_sub(var2, ps_s2, var2)
                    nc.scalar.activation(var2, var2, AF.Sqrt, bias=eps_ln[:, 0:1])
                    nc.vector.reciprocal_approx_fast(out=var2, in_=var2)
                    yo_t = bk.tile([P, DO, HW_], F16, tag="yo", name=f"yo_{it}_{sp}")
                    for o in range(DO):
                        nc.vector.tensor_sub(yo_t[:, o, :], y2_t[:, o, cs], m2_sb)
                    for o in range(DO):
                        nc.vector.scalar_tensor_tensor(yo_t[:, o, :], yo_t[:, o, :],
                                                       pp[:, o, N2G:N2G + 1], var2,
                                                       OP.mult, OP.mult)
                    if use_n2b:
                        for o in range(DO):
                            nc.vector.tensor_scalar_add(yo_t[:, o, :],
                                                        yo_t[:, o, :],
                                                        pp[:, o, N2B:N2B + 1])
                    nc.sync.dma_start(yT[:, :, n0 + sp * HW_:n0 + (sp + 1) * HW_],
                                      yo_t)

            acc_tiles = {0: d_load(0)}
            if NTILES > 1:
                acc_tiles[1] = d_load(1)
            acc_cur = d_front_b(0, d_front_a(0, acc_tiles.pop(0)))
            acc_nxt = None
            y2_prev = None
            # d_back runs one slot late so its ACT/DVE chains overlap the
            # next tile's d_mid/d_ffn matmuls
            for it in range(NTILES):
                y1_cur = d_mid(it, acc_cur)
                if it + 2 < NTILES:
                    acc_tiles[it + 2] = d_load(it + 2)
                if it + 1 < NTILES:
                    acc_nxt = d_front_a(it + 1, acc_tiles.pop(it + 1))
                if y2_prev is not None:
                    d_back(it - 1, y2_prev)
                y2_cur = d_ffn(it, y1_cur)
                if it + 1 < NTILES:
                    acc_cur = d_front_b(it + 1, acc_nxt)
                y2_prev = y2_cur
            d_back(NTILES - 1, y2_prev, nsplit=2)

    nc.compile()
    return nc


def make_in_maps(inputs, n_cores=8):
    """Host-side preprocessing: fold constants, transpose, cast, shard."""
    x = np.asarray(inputs["x"], np.float32)
    B, N, D_ = x.shape
    dt = float(np.asarray(inputs["delta_t"]))

    def g(k):
        return np.asarray(inputs[k], np.float32)

    diff_w, diff_b = g("diff_w"), g("diff_b")
    tm_w1, tm_cb1 = g("tm_w1"), g("tm_cb1")
    tm_w2, tm_cb2 = g("tm_w2"), g("tm_cb2")

    pp = np.zeros((P, DO, NPARAM), np.float32)

    def put(i, v):
        pp[:, :, i] = v.reshape(DO, P).T

    put(C0, dt * diff_w[:, 0, 0])
    put(C1, dt * diff_w[:, 0, 1] + (1.0 - dt))
    put(C2, dt * diff_w[:, 0, 2])
    put(CB, dt * diff_b + g("lu_b2") + tm_cb2)
    put(T0, tm_w1[:, 0, 0])
    put(T1, tm_w1[:, 0, 1])
    put(T2, tm_w1[:, 0, 2])
    put(TCB1, tm_cb1)
    put(U0, tm_w2[:, 0, 0])
    put(U1, tm_w2[:, 0, 1])
    put(U2, tm_w2[:, 0, 2])
    put(TMG, g("tm_g"))
    put(TMB, g("tm_beta"))
    put(N1G, g("n1_g"))
    put(N1B, g("n1_b"))
    put(N2G, g("n2_g"))
    put(N2B, g("n2_b"))
    put(LUB1, g("lu_b1"))
    put(FFB1, g("ff_b1"))
    put(FFB2, g("ff_b2"))

    diags = np.zeros((P, 3, DO, P), np.float32)
    idx = np.arange(P)
    for tap in range(3):
        for dc in range(DO):
            diags[idx, tap, dc, idx] = tm_w2[dc * P + idx, 0, tap]
    diags = diags.astype(F16_NP)

    rows = np.zeros((1, 3 * D), np.float32)
    rows[0, 0:D] = g("bq") * WS
    rows[0, D:2 * D] = g("bk") * KS
    rows[0, 2 * D:3 * D] = g("bv") * WS
    rows = rows.astype(BF16_NP)

    wt = {}
    for name, key in (("w1T", "lu_w1"), ("w2T", "lu_w2")):
        wt[name] = np.ascontiguousarray(g(key).T).astype(F16_NP)
    w8 = {}
    for name, key, s in (("wq8", "wq", WS), ("wk8", "wk", KS), ("wv8", "wv", WS),
                         ("f18", "ff_w1", WS), ("f28", "ff_w2", WS)):
        w8[name] = np.ascontiguousarray(g(key).T * s).astype(F8_NP)

    xT = np.ascontiguousarray(x.transpose(0, 2, 1)).astype(F16_NP)
    x8 = xT.astype(F8_NP)

    flags = dict(
        use_bq=bool(np.any(g("bq"))),
        use_bk=bool(np.any(g("bk"))),
        use_bv=bool(np.any(g("bv"))),
        use_tmb=bool(np.any(g("tm_beta"))),
        use_n1b=bool(np.any(g("n1_b"))),
        use_n2b=bool(np.any(g("n2_b"))),
        use_fb2=bool(np.any(g("ff_b2"))),
    )

    shared = {**wt, **w8, "pp": pp, "rows": rows, "diags": diags}
    in_maps = [{**shared, "x_T": xT[b], "x_8": x8[b]} for b in range(B)]
    return in_maps, flags, (B, N)


_NC_CACHE = {}


def kernel(**inputs):
    in_maps, flags, (B, N) = make_in_maps(inputs)
    key = (N, tuple(sorted(flags.items())))
    if key not in _NC_CACHE:
        _NC_CACHE[key] = build_nc(N=N, NT=512, **flags)
    nc = _NC_CACHE[key]
    res = run_bass_kernel_spmd(nc, in_maps, list(range(B)))
    y = np.stack([res.results[b]["y_T"] for b in range(B)])
    return np.ascontiguousarray(y.transpose(0, 2, 1)).astype(np.float32)
